# revision 21
# baseline (speedup 1.0000x reference)
"""Trainium2 Bass kernel for nn_EulerFullAttention.

Math (per batch b, head h, dh=64):
  theta_q = x/(1+|w_q|) + b_q + t*phi_q ; Q = [cos(theta_q), sin(theta_q)]  (S,128)
  theta_k likewise ; K = [cos, sin]
  V = cos(theta_v)+sin(theta_v) = sqrt(2)*sin(theta_v + pi/4)              (S,64)
  scores = Q @ K^T / sqrt(128), causal softmax, out = attn @ V
  result = cos(theta_o)+sin(theta_o) = sqrt(2)*sin(theta_o + pi/4)

Distribution: 8 cores = 2 batches x 4 head-groups (4 heads each). Each core
computes its x[:, 256-col] slice end to end; no collectives.

Trig via a custom DVE op FRAC_AFFINE_ANT: f = a - ((a + C) - C) with
C = 1.5*2^23 rounds a = in*s + c to nearest in one instruction, so
sin(theta) = Sin(2*pi*f) with f in [-0.5, 0.5]. Cos rows get +0.25 in c.

qk prep: x's 64 head-features are duplicated into both partition halves
(x2, gpsimd copies), PE-transposed per 128-block into PSUM, and a rank-2
f32r matmul [phi'; c'] @ [t; 1] accumulates the t*phi_q/(2pi*s)+c phase
(divided by the x scale s on host). One FRAC(in*s) + Sin(bf16 out) per
projection; the k projection reuses the same PSUM via a second rank-2
matmul adding the (k - q) phase delta.

Attention in transposed layout: scoresT[k, q] = KT.T @ QT, all-bf16
matmuls (1 cyc/row at any width). exp via ACT from PSUM -> bf16 SBUF;
causal via block structure + affine_select on diagonal blocks. attn@V
accumulates outT[65, 512] per 512-wide q chunk with lhsT = [V/sqrt2*...
actually [sv | 1/sqrt2] where sv = V/sqrt2, so row 64 is rowsum/sqrt2 and
one tensor_tensor divide per (head, chunk) normalizes after a PE
transpose back to natural layout.

ACT instruction stream is strictly Sin-phase, Exp-phase, Sin-phase so
only 3 activation-table loads are inserted.
"""

import sys, math

sys.path.insert(0, "/opt/trn_rl_repo")

import numpy as np
import concourse.bass as bass
import concourse.mybir as mybir
import concourse.dve_ops as dve_ops
from concourse.dve_ops import DveOp
from concourse.dve_spec import Spec, Src0, C0, C1, C2, lower as dve_lower
from concourse.dve_uop import DveOpSpec
from concourse.bacc import Bacc
from concourse.tile import TileContext
from concourse.bass_utils import run_bass_kernel_spmd
from contextlib import ExitStack

F32 = mybir.dt.float32
F32R = mybir.dt.float32r
BF16 = mybir.dt.bfloat16
AF = mybir.ActivationFunctionType
ALU = mybir.AluOpType

B, S, D, H = 2, 2048, 1024, 16
DH = 64
NH = 4            # heads per core
DC = NH * DH      # 256 feature columns per core
NB = S // 128     # 16 s-blocks
TWO_PI = 2.0 * math.pi
SQRT2 = math.sqrt(2.0)
INV_SQRT2 = 1.0 / SQRT2
EXP_SCALE = 1.0 / math.sqrt(2.0 * DH)
MAGIC = 12582912.0  # 1.5 * 2^23: (a + MAGIC) - MAGIC == round-to-nearest(a)
# out-stage chunks (block0, nblocks): small tail chunks shorten the
# last-exp -> last-DMA critical chain
OUT_CHUNKS = [(0, 4), (4, 4), (8, 4), (12, 2), (14, 2)]


# ---------------- custom DVE op: f = frac_rn(in0*s0 + s1) ----------------
def _frac_ref(in0, in1, s0, s1, imm2):
    a = np.float32(np.float32(in0 * np.float32(s0)) + np.float32(s1))
    t = np.float32(a + np.float32(imm2))
    u = np.float32(t - np.float32(imm2))
    return np.float32(a - u)


_fa = Src0 * C0 + C1
_FRAC_SPEC = Spec(body=_fa - ((_fa + C2) - C2), reference=_frac_ref)


def _register_frac_op():
    name = "FRAC_AFFINE_ANT"
    for op in dve_ops.OPS:
        if op.name == name:
            return op
    row = max(dve_ops._SUB_OPCODE_FOR_NAME.values()) + 1
    assert row < 0x20
    dve_ops._SUB_OPCODE_FOR_NAME[name] = row
    shas = {}
    for ver in ("v3", "v4"):
        spec_c = DveOpSpec(name=name, opcode=row,
                           uops=dve_lower(_FRAC_SPEC, ver=ver), rd1_en=False)
        shas[ver] = spec_c.sha(ver)
    op = DveOp(name, _FRAC_SPEC, subdim=False, uops_sha=shas)
    dve_ops.OPS.append(op)
    dve_ops.CUSTOM_DVE_SPECS[name] = _FRAC_SPEC
    return op


FRAC = _register_frac_op()


def _dup_mid(ap2d, n):
    """[128, F] AP -> [128, n, F] with stride-0 middle dim."""
    return bass.AP(tensor=ap2d.tensor, offset=ap2d.offset,
                   ap=[ap2d.ap[0], [0, n], ap2d.ap[-1]])


def _bcast_inner(ap2d, n):
    """[128, F] AP -> [128, F, n] with stride-0 inner dim."""
    return bass.AP(tensor=ap2d.tensor, offset=ap2d.offset,
                   ap=[ap2d.ap[0], ap2d.ap[-1], [0, n]])


def _build_packs(qc):
    """PSUM pack layout for one 512-wide q chunk: list of packs, each a list
    of (kb, qs, N, off) strips placed in a [128,1024] (2-bank) psum tile."""
    order = list(range(4 * qc)) + [4 * qc, 4 * qc + 1, 4 * qc + 3, 4 * qc + 2]
    packs, cur, off = [], [], 0
    for kb in order:
        if kb < 4 * qc:
            qs, N = 512 * qc, 512
        else:
            jj = kb - 4 * qc
            qs, N = 512 * qc + 128 * jj, 512 - 128 * jj
        o = off
        if o % 512 + N > 512:
            o = (o // 512 + 1) * 512
        if o + N > 1024:
            packs.append(cur)
            cur, o = [], 0
        cur.append((kb, qs, N, o))
        off = o + N
    if cur:
        packs.append(cur)
    return packs


def build_nc(c_v=0.125, c_o=0.125):
    """c_v / c_o: host-folded (b/2pi + 0.125) constants."""
    nc = Bacc(trn_type="TRN2")
    xin = nc.dram_tensor("xin", [S, DC], F32, kind="ExternalInput")
    sqk_d = nc.dram_tensor("sqk", [128, NH, 2], F32, kind="ExternalInput")
    phc_d = nc.dram_tensor("phc", [3, NH, 2, 128], F32, kind="ExternalInput")
    vp_d = nc.dram_tensor("vp", [128, DC], F32, kind="ExternalInput")
    op_d = nc.dram_tensor("opar", [128, DC], F32, kind="ExternalInput")
    out_d = nc.dram_tensor("out", [S, DC], F32, kind="ExternalOutput")
    ident_d = nc.inline_tensor(np.eye(128, dtype=np.float32), "ident")
    trow_np = np.stack([np.arange(S, dtype=np.float32),
                        np.arange(S, dtype=np.float32),
                        np.ones(S, dtype=np.float32)])
    trow_d = nc.inline_tensor(trow_np, "trow")

    with TileContext(nc) as tc, ExitStack() as ctx:
        sing = ctx.enter_context(tc.tile_pool(name="sing", bufs=1))
        qkpool = ctx.enter_context(tc.tile_pool(name="qkp", bufs=8))
        x2pool = ctx.enter_context(tc.tile_pool(name="x2p", bufs=2))
        mid = ctx.enter_context(tc.tile_pool(name="mid", bufs=3))
        otpool = ctx.enter_context(tc.tile_pool(name="otp", bufs=2))
        expool = ctx.enter_context(tc.tile_pool(name="exp", bufs=4))
        rvpool = ctx.enter_context(tc.tile_pool(name="rvp", bufs=2))
        svpool = ctx.enter_context(tc.tile_pool(name="svp", bufs=2))
        tiny = ctx.enter_context(tc.tile_pool(name="tiny", bufs=4))
        ropool = ctx.enter_context(tc.tile_pool(name="rop", bufs=2))
        pprep = ctx.enter_context(tc.tile_pool(name="pprep", bufs=2, space="PSUM"))
        psp = ctx.enter_context(tc.tile_pool(name="psp", bufs=2, space="PSUM"))
        pso = ctx.enter_context(tc.tile_pool(name="pso", bufs=1, space="PSUM"))
        psn = ctx.enter_context(tc.tile_pool(name="psn", bufs=1, space="PSUM"))

        # ---- params first (small), then x: keeps the single SP DMA queue
        # from delaying the prep-critical tensors.
        sqk = sing.tile([128, NH, 2], F32)
        nc.sync.dma_start(out=sqk, in_=sqk_d[:, :, :])
        vp = sing.tile([128, DC], F32)
        nc.sync.dma_start(out=vp, in_=vp_d[:, :])
        opr = sing.tile([128, DC], F32)
        nc.sync.dma_start(out=opr, in_=op_d[:, :])
        phc = sing.tile([3, NH, 2, 128], F32)
        nc.sync.dma_start(out=phc, in_=phc_d[:, :, :, :])
        trow = sing.tile([3, S], F32)
        nc.sync.dma_start(out=trow, in_=trow_d[:, :])
        ident = sing.tile([128, 128], F32)
        nc.sync.dma_start(out=ident, in_=ident_d[:, :])
        x_s = sing.tile([128, NB, DC], F32)
        xin_r = xin[:, :].rearrange("(n p) d -> p n d", p=128)
        for qq in range(4):
            nc.sync.dma_start(out=x_s[:, 4 * qq:4 * qq + 4, :],
                              in_=xin_r[:, 4 * qq:4 * qq + 4, :])

        bz = sing.tile([128, 1], F32)
        nc.vector.memset(bz, 0.0)
        onat = sing.tile([128, NB, DC], F32)
        vaug = []
        for j in range(NH):
            t = sing.tile([128, NB, DH + 1], BF16, tag=f"vaug{j}")
            nc.vector.memset(t[:, :, DH:DH + 1], INV_SQRT2)
            vaug.append(t)
        # f32r copies for the rank-2 phase matmul (fp32r inputs must be
        # engine-rounded, a bitcast does not satisfy the verifier)
        phcr = sing.tile([3, NH, 2, 128], F32R)
        nc.vector.tensor_copy(out=phcr, in_=phc)
        trowr = sing.tile([3, S], F32R)
        nc.vector.tensor_copy(out=trowr, in_=trow)


        QT, KT = [None] * NH, [None] * NH

        # ---------------- V (4 quarters) ----------------
        sv_tiles = []

        def v_quarter(qq):
            xh = x_s[:, 4 * qq:4 * qq + 4, :]
            rv = rvpool.tile([128, 4, DC], F32, tag="rv")
            nc.gpsimd.tensor_tensor(out=rv, in0=xh, in1=_dup_mid(vp[:, :], 4),
                                    op=ALU.mult)
            mv = rvpool.tile([128, 4, DC], F32, tag="rv")
            nc.vector._custom_dve(FRAC, out=mv, in0=rv, s0=1.0, s1=c_v,
                                  imm2=MAGIC)
            sv = svpool.tile([128, 4, DC], BF16, tag="sv")
            nc.scalar.activation(out=sv, in_=mv, func=AF.Sin,
                                 bias=bz[:, 0:1], scale=TWO_PI)
            sv_tiles.append((qq, sv))

        def v_scatter(qq, sv):
            for j in range(NH):
                nc.vector.tensor_copy(out=vaug[j][:, 4 * qq:4 * qq + 4, 0:DH],
                                      in_=sv[:, :, DH * j:DH * j + DH])

        # ---------------- QK prep ----------------
        def x2_dup(j):
            """x2[:, n, 0:64] = x2[:, n, 64:128] = x_s[:, n, 64j:64j+64]."""
            x2 = x2pool.tile([128, NB, 128], F32, tag="x2")
            for half in range(2):
                nc.gpsimd.tensor_copy(
                    out=x2[:, :, 64 * half:64 * half + 64],
                    in_=x_s[:, :, DH * j:DH * j + DH])
            return x2

        def qk_prep(j, x2):
            """Per cc: transposes + rank-2 q phase; FRAC + Sin per proj, with
            a rank-2 (k - q) delta matmul retargeting the PSUM to k."""
            mq = mid.tile([128, S], F32, tag="mid")
            mk = mid.tile([128, S], F32, tag="mid")
            tiles = []
            for cc in range(4):
                xq = pprep.tile([128, 512], F32, tag="prep")
                sl = slice(512 * cc, 512 * cc + 512)
                nc.tensor.matmul(xq, phcr[:, j, 0, :], trowr[:, sl],
                                 start=True, stop=True)
                for sb in range(4):
                    n = 4 * cc + sb
                    nc.tensor.matmul(xq[:, 128 * sb:128 * sb + 128],
                                     x2[:, n, :], ident,
                                     is_transpose=True, start=False, stop=True,
                                     skip_group_check=True)
                nc.vector._custom_dve(FRAC, out=mq[:, sl], in0=xq,
                                      s0=sqk[:, j, 0:1], s1=0.0, imm2=MAGIC)
                tiles.append((xq, sl))
            for (xq, sl) in tiles:
                nc.tensor.matmul(xq, phcr[:, j, 1, :], trowr[:, sl],
                                 start=False, stop=True, skip_group_check=True)
                nc.vector._custom_dve(FRAC, out=mk[:, sl], in0=xq,
                                      s0=sqk[:, j, 1:2], s1=0.0, imm2=MAGIC)
            tq = qkpool.tile([128, S], BF16, tag="qk")
            nc.scalar.activation(out=tq, in_=mq, func=AF.Sin,
                                 bias=bz[:, 0:1], scale=TWO_PI)
            QT[j] = tq
            tk = qkpool.tile([128, S], BF16, tag="qk")
            nc.scalar.activation(out=tk, in_=mk, func=AF.Sin,
                                 bias=bz[:, 0:1], scale=TWO_PI)
            KT[j] = tk

        # ---------------- attention ----------------
        def attention(j):
            for qc in range(4):
                ot_ps = pso.tile([65, 512], F32, tag="po")
                packs = _build_packs(qc)
                n_av = 4 * qc + 4
                avi = 0
                for pack in packs:
                    sc = psp.tile([128, 1024], F32, tag="ps")
                    for (kb, qs, N, off) in pack:
                        nc.tensor.matmul(sc[:, off:off + N],
                                         KT[j][:, 128 * kb:128 * kb + 128],
                                         QT[j][:, qs:qs + N],
                                         start=True, stop=True)
                    width = pack[-1][3] + pack[-1][2]
                    ext = expool.tile([128, 1024], BF16, tag="ex")
                    nc.scalar.activation(out=ext[:, 0:width], in_=sc[:, 0:width],
                                         func=AF.Exp, bias=bz[:, 0:1],
                                         scale=EXP_SCALE)
                    for (kb, qs, N, off) in pack:
                        if kb >= 4 * qc:  # diagonal strip: zero exp where q < k
                            nc.gpsimd.affine_select(
                                out=ext[:, off:off + 128], in_=ext[:, off:off + 128],
                                pattern=[[1, 128]], compare_op=ALU.is_ge, fill=0.0,
                                base=0, channel_multiplier=-1)
                    for (kb, qs, N, off) in pack:
                        q0 = qs - 512 * qc
                        nc.tensor.matmul(ot_ps[:, q0:q0 + N],
                                         vaug[j][:, kb, :],
                                         ext[:, off:off + N],
                                         start=(avi == 0), stop=(avi == n_av - 1))
                        avi += 1
                ot_s = otpool.tile([65, 512], F32, tag="ot")
                nc.vector.tensor_copy(out=ot_s, in_=ot_ps)
                on_ps = psn.tile([128, 4, DH + 1], F32, tag="pn")
                for t4 in range(4):
                    nc.tensor.matmul(on_ps[:, t4, :],
                                     ot_s[:, 128 * t4:128 * t4 + 128],
                                     ident[0:65, 0:65],
                                     is_transpose=True, start=True, stop=True)
                den = tiny.tile([128, 4], F32, tag="tiny")
                nc.vector.reciprocal(out=den, in_=on_ps[:, :, DH:DH + 1])
                nc.vector.tensor_tensor(
                    out=onat[:, 4 * qc:4 * qc + 4, DH * j:DH * j + DH],
                    in0=on_ps[:, :, 0:DH], in1=_bcast_inner(den, DH),
                    op=ALU.mult)

        # ---------------- emission order ----------------
        for _q in range(4):
            v_quarter(_q)
        x2s = [x2_dup(j) for j in range(2)]
        for j in range(NH):
            if j + 2 < NH:
                x2s.append(x2_dup(j + 2))
            qk_prep(j, x2s[j])
        for qq, sv in sv_tiles:
            v_scatter(qq, sv)

        for j in range(NH):
            attention(j)

        # ---------------- final layer ----------------
        out_r = out_d[:, :].rearrange("(n p) d -> p n d", p=128)
        for (b0, nb) in OUT_CHUNKS:
            ro = ropool.tile([128, 4, DC], F32, tag="ro")
            rv = ro[:, 0:nb, :]
            nc.gpsimd.tensor_tensor(out=rv, in0=onat[:, b0:b0 + nb, :],
                                    in1=_dup_mid(opr[:, :], nb), op=ALU.mult)
            nc.vector._custom_dve(FRAC, out=rv, in0=rv, s0=1.0, s1=c_o,
                                  imm2=MAGIC)
            nc.scalar.activation(out=rv, in_=rv, func=AF.Sin,
                                 bias=bz[:, 0:1], scale=TWO_PI)
            nc.vector.tensor_scalar(out=rv, in0=rv, scalar1=SQRT2, scalar2=None,
                                    op0=ALU.mult)
            nc.sync.dma_start(out=out_r[:, b0:b0 + nb, :], in_=rv)

    nc.finalize()
    return nc


def _host_params(inputs, c):
    """Per-core input dict for core c."""
    b, g = c // 4, c % 4
    inv2pi = 1.0 / (2.0 * np.pi)
    x = np.asarray(inputs["x"], dtype=np.float32)
    xin = np.ascontiguousarray(x[b, :, DC * g:DC * g + DC])

    def f64(a):
        return np.asarray(a, dtype=np.float64)

    def hi_lo(v):
        """Split f64 v into f32 hi (12 mantissa bits, so hi*t is exact in
        f32r for t < 2^12) + f32 lo remainder."""
        m, e = np.frexp(v)
        hi = np.ldexp(np.round(m * 2**12) / 2**12, e).astype(np.float32)
        lo = (v - hi.astype(np.float64)).astype(np.float32)
        return hi, lo

    rows = np.arange(128) % DH
    cos_row = (np.arange(128) < DH).astype(np.float64) * 0.25
    sqk = np.zeros((128, NH, 2), dtype=np.float32)
    phc = np.zeros((3, NH, 2, 128), dtype=np.float32)
    for j in range(NH):
        h = NH * g + j
        ph_s = {}
        for pi, (wn, bn, pn) in enumerate([("w_q", "b_q", "phi_q"),
                                           ("w_k", "b_k", "phi_k")]):
            w = f64(inputs[wn])[h]
            bb = f64(inputs[bn])[h]
            ph = f64(inputs[pn])[h]
            s = (inv2pi / (1.0 + np.abs(w)))[rows]
            phi2 = (ph * inv2pi)[rows] / s
            c2 = ((bb * inv2pi)[rows] + cos_row) / s
            sqk[:, j, pi] = s
            ph_s[pi] = (phi2, c2)
        phq_hi, phq_lo = hi_lo(ph_s[0][0])
        phk_hi, _ = hi_lo(ph_s[1][0])
        dphi_hi = (phk_hi - phq_hi).astype(np.float64)
        dphi_lo = (ph_s[1][0] - ph_s[0][0] - dphi_hi).astype(np.float32)
        phc[0, j, 0, :] = phq_hi
        phc[1, j, 0, :] = phq_lo
        phc[2, j, 0, :] = ph_s[0][1]
        phc[0, j, 1, :] = dphi_hi                   # k - q phase delta
        phc[1, j, 1, :] = dphi_lo
        phc[2, j, 1, :] = ph_s[1][1] - ph_s[0][1]

    vp = np.zeros((128, DC), dtype=np.float32)
    wv = f64(inputs["w_v"])[NH * g:NH * g + NH].reshape(-1)
    vp[:, :] = (inv2pi / (1.0 + np.abs(wv)))[None, :]

    op = np.zeros((128, DC), dtype=np.float32)
    wo = f64(inputs["w_out"])[DC * g:DC * g + DC]
    op[:, :] = (inv2pi / (1.0 + np.abs(wo)))[None, :]

    return {"xin": xin, "sqk": sqk, "phc": phc, "vp": vp, "opar": op}


_NC_CACHE = {}


def kernel(**inputs) -> np.ndarray:
    in_maps = [_host_params(inputs, c) for c in range(8)]
    inv2pi = 1.0 / (2.0 * np.pi)
    bv = np.asarray(inputs["b_v"], dtype=np.float64).reshape(-1)
    bo = np.asarray(inputs["b_out"], dtype=np.float64).reshape(-1)
    assert np.all(bv == bv[0]) and np.all(bo == bo[0]), "non-uniform b_v/b_out unsupported"
    c_v = float(np.float32(bv[0] * inv2pi + 0.125))
    c_o = float(np.float32(bo[0] * inv2pi + 0.125))
    key = (c_v, c_o)
    if _NC_CACHE.get("key") != key:
        _NC_CACHE["nc"] = build_nc(c_v, c_o)
        _NC_CACHE["key"] = key
    nc = _NC_CACHE["nc"]
    res = run_bass_kernel_spmd(nc, in_maps, core_ids=list(range(8)))
    full = np.empty((B, S, D), dtype=np.float32)
    for c in range(8):
        b, g = c // 4, c % 4
        full[b, :, DC * g:DC * g + DC] = res.results[c]["out"]
    return full


# revision 53
# speedup vs baseline: 1.1244x; 1.1244x over previous
"""Trainium2 Bass kernel for nn_EulerFullAttention.

Math (per batch b, head h, dh=64):
  theta_q = x/(1+|w_q|) + b_q + t*phi_q ; Q = [cos(theta_q), sin(theta_q)]  (S,128)
  theta_k likewise ; K = [cos, sin]
  V = cos(theta_v)+sin(theta_v) = sqrt(2)*sin(theta_v + pi/4)              (S,64)
  scores = Q @ K^T / sqrt(128), causal softmax, out = attn @ V
  result = cos(theta_o)+sin(theta_o) = sqrt(2)*sin(theta_o + pi/4)

Distribution: 8 cores = 2 batches x 4 head-groups (4 heads each). Each core
computes its x[:, 256-col] slice end to end; no collectives.

Trig via a custom DVE op FRAC_AFFINE_ANT: f = a - ((a + C) - C) with
C = 1.5*2^23 rounds a = in*s + c to nearest in one instruction, so
sin(theta) = Sin(2*pi*f) with f in [-0.5, 0.5]. Cos rows get +0.25 in c.

qk prep: x's 64 head-features are duplicated into both partition halves
(x2, gpsimd copies), PE-transposed per 128-block into PSUM, and a rank-2
f32r matmul [phi'; c'] @ [t; 1] accumulates the t*phi_q/(2pi*s)+c phase
(divided by the x scale s on host). One FRAC(in*s) + Sin(bf16 out) per
projection; the k projection reuses the same PSUM via a second rank-2
matmul adding the (k - q) phase delta.

Attention in transposed layout: scoresT[k, q] = KT.T @ QT, all-bf16
matmuls (1 cyc/row at any width). exp via ACT from PSUM -> bf16 SBUF;
causal via block structure + affine_select on diagonal blocks. attn@V
accumulates outT[65, 512] per 512-wide q chunk with lhsT = [V/sqrt2*...
actually [sv | 1/sqrt2] where sv = V/sqrt2, so row 64 is rowsum/sqrt2 and
one tensor_tensor divide per (head, chunk) normalizes after a PE
transpose back to natural layout.

ACT instruction stream is strictly Sin-phase, Exp-phase, Sin-phase so
only 3 activation-table loads are inserted.
"""

import sys, math

sys.path.insert(0, "/opt/trn_rl_repo")

import numpy as np
import concourse.bass as bass
import concourse.mybir as mybir
import concourse.dve_ops as dve_ops
from concourse.dve_ops import DveOp
from concourse.dve_spec import Spec, Src0, Src1, C0, C1, C2, lower as dve_lower
from concourse.dve_uop import DveOpSpec
from concourse.bacc import Bacc
from concourse.tile import TileContext
from concourse.bass_utils import run_bass_kernel_spmd
from contextlib import ExitStack

F32 = mybir.dt.float32
F32R = mybir.dt.float32r
BF16 = mybir.dt.bfloat16
AF = mybir.ActivationFunctionType
ALU = mybir.AluOpType

B, S, D, H = 2, 2048, 1024, 16
DH = 64
NH = 4            # heads per core
DC = NH * DH      # 256 feature columns per core
NB = S // 128     # 16 s-blocks
TWO_PI = 2.0 * math.pi
SQRT2 = math.sqrt(2.0)
INV_SQRT2 = 1.0 / SQRT2
EXP_SCALE = 1.0 / math.sqrt(2.0 * DH)
MAGIC = 12582912.0  # 1.5 * 2^23: (a + MAGIC) - MAGIC == round-to-nearest(a)
# out-stage chunks (block0, nblocks): small tail chunks shorten the
# last-exp -> last-DMA critical chain
OUT_CHUNKS = [(0, 4), (4, 4), (8, 4), (12, 2), (14, 2)]


# ---------------- custom DVE op: f = frac_rn(in0*s0 + s1) ----------------
def _frac_ref(in0, in1, s0, s1, imm2):
    a = np.float32(np.float32(in0 * np.float32(s0)) + np.float32(s1))
    t = np.float32(a + np.float32(imm2))
    u = np.float32(t - np.float32(imm2))
    return np.float32(a - u)


def _frac2_ref(in0, in1, s0, s1, imm2):
    a = np.float32(np.float32(in0 * in1) + np.float32(s0))
    t = np.float32(a + np.float32(s1))
    u = np.float32(t - np.float32(s1))
    return np.float32(a - u)


_fa = Src0 * C0 + C1
_FRAC_SPEC = Spec(body=_fa - ((_fa + C2) - C2), reference=_frac_ref)
_f2 = Src0 * Src1 + C0
_FRAC2_SPEC = Spec(body=_f2 - ((_f2 + C1) - C1), reference=_frac2_ref)


def _register_op(name, spec, rd1):
    for op in dve_ops.OPS:
        if op.name == name:
            return op
    row = max(dve_ops._SUB_OPCODE_FOR_NAME.values()) + 1
    assert row < 0x20
    dve_ops._SUB_OPCODE_FOR_NAME[name] = row
    shas = {}
    for ver in ("v3", "v4"):
        spec_c = DveOpSpec(name=name, opcode=row,
                           uops=dve_lower(spec, ver=ver), rd1_en=rd1)
        shas[ver] = spec_c.sha(ver)
    op = DveOp(name, spec, subdim=False, uops_sha=shas)
    dve_ops.OPS.append(op)
    dve_ops.CUSTOM_DVE_SPECS[name] = spec
    return op


FRAC = _register_op("FRAC_AFFINE_ANT", _FRAC_SPEC, False)
FRAC2 = _register_op("FRAC_MUL_ANT", _FRAC2_SPEC, True)


def _dup_mid(ap2d, n):
    """[128, F] AP -> [128, n, F] with stride-0 middle dim."""
    return bass.AP(tensor=ap2d.tensor, offset=ap2d.offset,
                   ap=[ap2d.ap[0], [0, n], ap2d.ap[-1]])


def _bcast_inner(ap2d, n):
    """[128, F] AP -> [128, F, n] with stride-0 inner dim."""
    return bass.AP(tensor=ap2d.tensor, offset=ap2d.offset,
                   ap=[ap2d.ap[0], ap2d.ap[-1], [0, n]])


def _build_packs(qc):
    """PSUM pack layout for one 512-wide q chunk: list of packs, each a list
    of (kb, qs, N, off) strips placed in a [128,1024] (2-bank) psum tile."""
    order = list(range(4 * qc)) + [4 * qc, 4 * qc + 1, 4 * qc + 3, 4 * qc + 2]
    packs, cur, off = [], [], 0
    for kb in order:
        if kb < 4 * qc:
            qs, N = 512 * qc, 512
        else:
            jj = kb - 4 * qc
            qs, N = 512 * qc + 128 * jj, 512 - 128 * jj
        o = off
        if o % 512 + N > 512:
            o = (o // 512 + 1) * 512
        if o + N > 1024:
            packs.append(cur)
            cur, o = [], 0
        cur.append((kb, qs, N, o))
        off = o + N
    if cur:
        packs.append(cur)
    return packs


def build_nc(c_v=0.125, c_o=0.125):
    """c_v / c_o: host-folded (b/2pi + 0.125) constants."""
    nc = Bacc(trn_type="TRN2")
    xin = nc.dram_tensor("xin", [S, DC], F32, kind="ExternalInput")
    sqk_d = nc.dram_tensor("sqk", [128, NH, 2], F32, kind="ExternalInput")
    phc_d = nc.dram_tensor("phc", [3, NH, 2, 128], F32, kind="ExternalInput")
    vp_d = nc.dram_tensor("vp", [128, DC], F32, kind="ExternalInput")
    op_d = nc.dram_tensor("opar", [128, DC], F32, kind="ExternalInput")
    out_d = nc.dram_tensor("out", [S, DC], F32, kind="ExternalOutput")
    ident_d = nc.inline_tensor(np.eye(128, dtype=np.float32), "ident")
    trow_np = np.stack([np.arange(S, dtype=np.float32),
                        np.arange(S, dtype=np.float32),
                        np.ones(S, dtype=np.float32)])
    trow_d = nc.inline_tensor(trow_np, "trow")

    with TileContext(nc) as tc, ExitStack() as ctx:
        sing = ctx.enter_context(tc.tile_pool(name="sing", bufs=1))
        qkpool = ctx.enter_context(tc.tile_pool(name="qkp", bufs=8))
        x2pool = ctx.enter_context(tc.tile_pool(name="x2p", bufs=2))
        mid = ctx.enter_context(tc.tile_pool(name="mid", bufs=3))
        otpool = ctx.enter_context(tc.tile_pool(name="otp", bufs=2))
        expool = ctx.enter_context(tc.tile_pool(name="exp", bufs=4))
        rvpool = ctx.enter_context(tc.tile_pool(name="rvp", bufs=2))
        svpool = ctx.enter_context(tc.tile_pool(name="svp", bufs=2))
        tiny = ctx.enter_context(tc.tile_pool(name="tiny", bufs=4))
        ropool = ctx.enter_context(tc.tile_pool(name="rop", bufs=5))
        pprep = ctx.enter_context(tc.tile_pool(name="pprep", bufs=2, space="PSUM"))
        psp = ctx.enter_context(tc.tile_pool(name="psp", bufs=2, space="PSUM"))
        pso = ctx.enter_context(tc.tile_pool(name="pso", bufs=1, space="PSUM"))
        psn = ctx.enter_context(tc.tile_pool(name="psn", bufs=1, space="PSUM"))

        # ---- two parallel HWDGE queues: x quarters 0-2 on the ACT queue,
        # params + x quarter 3 on the SP queue.
        x_s = sing.tile([128, NB, DC], F32)
        xin_r = xin[:, :].rearrange("(n p) d -> p n d", p=128)
        for qq in range(3):
            nc.scalar.dma_start(out=x_s[:, 4 * qq:4 * qq + 4, :],
                                in_=xin_r[:, 4 * qq:4 * qq + 4, :])
        vp = sing.tile([128, DC], F32)
        nc.sync.dma_start(out=vp, in_=vp_d[:, :])
        sqk = sing.tile([128, NH, 2], F32)
        nc.sync.dma_start(out=sqk, in_=sqk_d[:, :, :])
        ident = sing.tile([128, 128], F32)
        nc.sync.dma_start(out=ident, in_=ident_d[:, :])
        phcr = sing.tile([3, NH, 2, 128], F32R)
        nc.sync.dma_start(out=phcr, in_=phc_d[:, :, :, :].bitcast(F32R))
        trowr = sing.tile([3, S], F32R)
        nc.sync.dma_start(out=trowr, in_=trow_d[:, :].bitcast(F32R))
        nc.sync.dma_start(out=x_s[:, 12:16, :], in_=xin_r[:, 12:16, :])
        opr = sing.tile([128, DC], F32)
        nc.sync.dma_start(out=opr, in_=op_d[:, :])

        bz = sing.tile([128, 1], F32)
        nc.vector.memset(bz, 0.0)
        onat = sing.tile([128, NB, DC], F32)
        vaug = []
        for j in range(NH):
            t = sing.tile([128, NB, DH + 1], BF16, tag=f"vaug{j}")
            nc.vector.memset(t[:, :, DH:DH + 1], INV_SQRT2)
            vaug.append(t)



        QT, KT = [None] * NH, [None] * NH
        # ACT-stream phase tracking: the Tile scheduler reorders freely and
        # does not model activation-table loads; explicit deps pin a
        # 5-phase order (sins v+h0, exps h0, sins h1-3, exps h1-3, out sins)
        # so head 0's exps start as soon as its Q/K are ready.
        sins, exps, osins = [], [], []

        # ---------------- V (4 quarters) ----------------
        sv_tiles = []

        def v_quarter(qq):
            xh = x_s[:, 4 * qq:4 * qq + 4, :]
            mv = rvpool.tile([128, 4, DC], F32, tag="rv")
            nc.vector._custom_dve(FRAC2, out=mv, in0=xh,
                                  in1=_dup_mid(vp[:, :], 4), s0=c_v, s1=MAGIC)
            sv = svpool.tile([128, 4, DC], BF16, tag="sv")
            sins.append(nc.scalar.activation(out=sv, in_=mv, func=AF.Sin,
                                             bias=bz[:, 0:1], scale=TWO_PI))
            sv_tiles.append((qq, sv))

        def v_scatter(qq, sv):
            for j in range(NH):
                nc.vector.tensor_copy(out=vaug[j][:, 4 * qq:4 * qq + 4, 0:DH],
                                      in_=sv[:, :, DH * j:DH * j + DH])

        # ---------------- QK prep ----------------
        def x2_dup(j, quarters=(slice(0, NB),)):
            """x2[:, n, 0:64] = x2[:, n, 64:128] = x_s[:, n, 64j:64j+64]."""
            x2 = x2pool.tile([128, NB, 128], F32, tag="x2")
            for qs in quarters:
                for half in range(2):
                    nc.gpsimd.tensor_copy(
                        out=x2[:, qs, 64 * half:64 * half + 64],
                        in_=x_s[:, qs, DH * j:DH * j + DH])
            return x2

        def qk_prep(j, x2, mid_cb=None):
            """Per cc: rank-2 q phase, then transposes of the dup'd x block;
            FRAC + Sin per proj, with a rank-2 (k - q) delta matmul
            retargeting the PSUM to k. mid_cb emits between the q and k
            halves (DVE-stream ordering control)."""
            mq = mid.tile([128, S], F32, tag="mid")
            mk = mid.tile([128, S], F32, tag="mid")
            tiles = []
            for cc in range(4):
                xq = pprep.tile([128, 512], F32, tag="prep")
                sl = slice(512 * cc, 512 * cc + 512)
                nc.tensor.matmul(xq, phcr[:, j, 0, :], trowr[:, sl],
                                 start=True, stop=True)
                for sb in range(4):
                    n = 4 * cc + sb
                    nc.tensor.matmul(xq[:, 128 * sb:128 * sb + 128],
                                     x2[:, n, :], ident,
                                     is_transpose=True, start=False, stop=True,
                                     skip_group_check=True)
                nc.vector._custom_dve(FRAC, out=mq[:, sl], in0=xq,
                                      s0=sqk[:, j, 0:1], s1=0.0, imm2=MAGIC)
                tiles.append((xq, sl))
            if mid_cb is not None:
                mid_cb()
            for (xq, sl) in tiles:
                nc.tensor.matmul(xq, phcr[:, j, 1, :], trowr[:, sl],
                                 start=False, stop=True, skip_group_check=True)
                nc.vector._custom_dve(FRAC, out=mk[:, sl], in0=xq,
                                      s0=sqk[:, j, 1:2], s1=0.0, imm2=MAGIC)
            tq = qkpool.tile([128, S], BF16, tag="qk")
            sins.append(nc.scalar.activation(out=tq, in_=mq, func=AF.Sin,
                                             bias=bz[:, 0:1], scale=TWO_PI))
            QT[j] = tq
            tk = qkpool.tile([128, S], BF16, tag="qk")
            sins.append(nc.scalar.activation(out=tk, in_=mk, func=AF.Sin,
                                             bias=bz[:, 0:1], scale=TWO_PI))
            KT[j] = tk

        # ---------------- attention ----------------
        def attention(j):
            for qc in range(4):
                ot_ps = pso.tile([65, 512], F32, tag="po")
                packs = _build_packs(qc)
                n_av = 4 * qc + 4
                avi = 0
                for pack in packs:
                    sc = psp.tile([128, 1024], F32, tag="ps")
                    for (kb, qs, N, off) in pack:
                        nc.tensor.matmul(sc[:, off:off + N],
                                         KT[j][:, 128 * kb:128 * kb + 128],
                                         QT[j][:, qs:qs + N],
                                         start=True, stop=True)
                    width = pack[-1][3] + pack[-1][2]
                    ext = expool.tile([128, 1024], BF16, tag="ex")
                    exps.append(nc.scalar.activation(
                        out=ext[:, 0:width], in_=sc[:, 0:width],
                        func=AF.Exp, bias=bz[:, 0:1], scale=EXP_SCALE))
                    for (kb, qs, N, off) in pack:
                        if kb >= 4 * qc:  # diagonal strip: zero exp where q < k
                            nc.gpsimd.affine_select(
                                out=ext[:, off:off + 128], in_=ext[:, off:off + 128],
                                pattern=[[1, 128]], compare_op=ALU.is_ge, fill=0.0,
                                base=0, channel_multiplier=-1)
                    for (kb, qs, N, off) in pack:
                        q0 = qs - 512 * qc
                        nc.tensor.matmul(ot_ps[:, q0:q0 + N],
                                         vaug[j][:, kb, :],
                                         ext[:, off:off + N],
                                         start=(avi == 0), stop=(avi == n_av - 1))
                        avi += 1
                ot_s = otpool.tile([65, 512], F32, tag="ot")
                nc.vector.tensor_copy(out=ot_s, in_=ot_ps)
                on_ps = psn.tile([128, 4, DH + 1], F32, tag="pn")
                for t4 in range(4):
                    nc.tensor.matmul(on_ps[:, t4, :],
                                     ot_s[:, 128 * t4:128 * t4 + 128],
                                     ident[0:65, 0:65],
                                     is_transpose=True, start=True, stop=True)
                den = tiny.tile([128, 4], F32, tag="tiny")
                nc.vector.reciprocal(out=den, in_=on_ps[:, :, DH:DH + 1])
                nc.vector.tensor_tensor(
                    out=onat[:, 4 * qc:4 * qc + 4, DH * j:DH * j + DH],
                    in0=on_ps[:, :, 0:DH], in1=_bcast_inner(den, DH),
                    op=ALU.mult)

        # ---------------- emission order ----------------
        # Pool: head-0 x2 copies per quarter as x lands, then later heads'.
        # DVE: v FRAC2s first (x-gated), then head-0 FRACs.
        v_quarter(0)
        v_quarter(1)
        v_quarter(2)
        x2_0 = x2_dup(0, quarters=[slice(4 * q, 4 * q + 4) for q in range(4)])
        qk_prep(0, x2_0, mid_cb=lambda: v_quarter(3))
        for qq, sv in sv_tiles:
            v_scatter(qq, sv)
        qk_prep(1, x2_dup(1))
        attention(0)
        nsin_a = 6            # v + head-0 sins: phase A
        nexp_h0 = len(exps)
        qk_prep(2, x2_dup(2))
        attention(1)
        qk_prep(3, x2_dup(3))
        attention(2)
        attention(3)

        # ---------------- final layer ----------------
        out_r = out_d[:, :].rearrange("(n p) d -> p n d", p=128)
        ro_chunks = []
        for (b0, nb) in OUT_CHUNKS:
            ro = ropool.tile([128, 4, DC], F32, tag="ro")
            rv = ro[:, 0:nb, :]
            nc.vector._custom_dve(FRAC2, out=rv, in0=onat[:, b0:b0 + nb, :],
                                  in1=_dup_mid(opr[:, :], nb), s0=c_o, s1=MAGIC)
            ro_chunks.append(rv)
        for ci, (b0, nb) in enumerate(OUT_CHUNKS):
            rv = ro_chunks[ci]
            osins.append(nc.scalar.activation(out=rv, in_=rv, func=AF.Sin,
                                              bias=bz[:, 0:1], scale=TWO_PI))
            nc.vector.tensor_scalar(out=rv, in0=rv, scalar1=SQRT2, scalar2=None,
                                    op0=ALU.mult)
            dma_eng = nc.sync if ci % 2 == 0 else nc.scalar
            dma_eng.dma_start(out=out_r[:, b0:b0 + nb, :], in_=rv)

        # phase-order bridges (A < B < C < D < E):
        #   A: v + h0 sins, B: h0 exps, C: h1-3 sins, D: h1-3 exps, E: out sins
        def bridge(prev_phase, next_phase):
            first = next_phase[0]
            for p_i in prev_phase:
                bass._add_dep_helper(first.ins, p_i.ins, sync=True,
                                     reason="act-table-order")
            for n_i in next_phase[1:]:
                bass._add_dep_helper(n_i.ins, first.ins, sync=True,
                                     reason="act-table-order")

        sins_a, sins_c = sins[:nsin_a], sins[nsin_a:]
        exps_b, exps_d = exps[:nexp_h0], exps[nexp_h0:]
        bridge(sins_a, exps_b)
        bridge(exps_b, sins_c)
        bridge(sins_c, exps_d)
        bridge(exps_d, osins)

    nc.finalize()
    return nc


def _host_params(inputs, c):
    """Per-core input dict for core c."""
    b, g = c // 4, c % 4
    inv2pi = 1.0 / (2.0 * np.pi)
    x = np.asarray(inputs["x"], dtype=np.float32)
    xin = np.ascontiguousarray(x[b, :, DC * g:DC * g + DC])

    def f64(a):
        return np.asarray(a, dtype=np.float64)

    def hi_lo(v):
        """Split f64 v into f32 hi (12 mantissa bits, so hi*t is exact in
        f32r for t < 2^12) + f32 lo remainder."""
        m, e = np.frexp(v)
        hi = np.ldexp(np.round(m * 2**12) / 2**12, e).astype(np.float32)
        lo = (v - hi.astype(np.float64)).astype(np.float32)
        return hi, lo

    rows = np.arange(128) % DH
    cos_row = (np.arange(128) < DH).astype(np.float64) * 0.25
    sqk = np.zeros((128, NH, 2), dtype=np.float32)
    phc = np.zeros((3, NH, 2, 128), dtype=np.float32)
    for j in range(NH):
        h = NH * g + j
        ph_s = {}
        for pi, (wn, bn, pn) in enumerate([("w_q", "b_q", "phi_q"),
                                           ("w_k", "b_k", "phi_k")]):
            w = f64(inputs[wn])[h]
            bb = f64(inputs[bn])[h]
            ph = f64(inputs[pn])[h]
            s = (inv2pi / (1.0 + np.abs(w)))[rows]
            phi2 = (ph * inv2pi)[rows] / s
            c2 = ((bb * inv2pi)[rows] + cos_row) / s
            sqk[:, j, pi] = s
            ph_s[pi] = (phi2, c2)
        phq_hi, phq_lo = hi_lo(ph_s[0][0])
        phk_hi, _ = hi_lo(ph_s[1][0])
        dphi_hi = (phk_hi - phq_hi).astype(np.float64)
        dphi_lo = (ph_s[1][0] - ph_s[0][0] - dphi_hi).astype(np.float32)
        phc[0, j, 0, :] = phq_hi
        phc[1, j, 0, :] = phq_lo
        phc[2, j, 0, :] = ph_s[0][1]
        phc[0, j, 1, :] = dphi_hi                   # k - q phase delta
        phc[1, j, 1, :] = dphi_lo
        phc[2, j, 1, :] = ph_s[1][1] - ph_s[0][1]

    vp = np.zeros((128, DC), dtype=np.float32)
    wv = f64(inputs["w_v"])[NH * g:NH * g + NH].reshape(-1)
    vp[:, :] = (inv2pi / (1.0 + np.abs(wv)))[None, :]

    op = np.zeros((128, DC), dtype=np.float32)
    wo = f64(inputs["w_out"])[DC * g:DC * g + DC]
    op[:, :] = (inv2pi / (1.0 + np.abs(wo)))[None, :]

    return {"xin": xin, "sqk": sqk, "phc": phc, "vp": vp, "opar": op}


_NC_CACHE = {}


def kernel(**inputs) -> np.ndarray:
    in_maps = [_host_params(inputs, c) for c in range(8)]
    inv2pi = 1.0 / (2.0 * np.pi)
    bv = np.asarray(inputs["b_v"], dtype=np.float64).reshape(-1)
    bo = np.asarray(inputs["b_out"], dtype=np.float64).reshape(-1)
    assert np.all(bv == bv[0]) and np.all(bo == bo[0]), "non-uniform b_v/b_out unsupported"
    c_v = float(np.float32(bv[0] * inv2pi + 0.125))
    c_o = float(np.float32(bo[0] * inv2pi + 0.125))
    key = (c_v, c_o)
    if _NC_CACHE.get("key") != key:
        _NC_CACHE["nc"] = build_nc(c_v, c_o)
        _NC_CACHE["key"] = key
    nc = _NC_CACHE["nc"]
    res = run_bass_kernel_spmd(nc, in_maps, core_ids=list(range(8)))
    full = np.empty((B, S, D), dtype=np.float32)
    for c in range(8):
        b, g = c // 4, c % 4
        full[b, :, DC * g:DC * g + DC] = res.results[c]["out"]
    return full


# revision 87
# speedup vs baseline: 1.1590x; 1.0308x over previous
"""Trainium2 Bass kernel for nn_EulerFullAttention.

Math (per batch b, head h, dh=64):
  theta_q = x/(1+|w_q|) + b_q + t*phi_q ; Q = [cos(theta_q), sin(theta_q)]  (S,128)
  theta_k likewise ; K = [cos, sin]
  V = cos(theta_v)+sin(theta_v) = sqrt(2)*sin(theta_v + pi/4)              (S,64)
  scores = Q @ K^T / sqrt(128), causal softmax, out = attn @ V
  result = cos(theta_o)+sin(theta_o) = sqrt(2)*sin(theta_o + pi/4)

Distribution: 8 cores = 2 batches x 4 head-groups (4 heads each). Each core
computes its x[:, 256-col] slice end to end; no collectives.

Trig via a custom DVE op FRAC_AFFINE_ANT: f = a - ((a + C) - C) with
C = 1.5*2^23 rounds a = in*s + c to nearest in one instruction, so
sin(theta) = Sin(2*pi*f) with f in [-0.5, 0.5]. Cos rows get +0.25 in c.

qk prep: x's 64 head-features are duplicated into both partition halves
(x2, gpsimd copies), PE-transposed per 128-block into PSUM, and a rank-2
f32r matmul [phi'; c'] @ [t; 1] accumulates the t*phi_q/(2pi*s)+c phase
(divided by the x scale s on host). One FRAC(in*s) + Sin(bf16 out) per
projection; the k projection reuses the same PSUM via a second rank-2
matmul adding the (k - q) phase delta.

Attention in transposed layout: scoresT[k, q] = KT.T @ QT, all-bf16
matmuls (1 cyc/row at any width). exp via ACT from PSUM -> bf16 SBUF;
causal via block structure + affine_select on diagonal blocks. attn@V
accumulates outT[65, 512] per 512-wide q chunk with lhsT = [V/sqrt2*...
actually [sv | 1/sqrt2] where sv = V/sqrt2, so row 64 is rowsum/sqrt2 and
one tensor_tensor divide per (head, chunk) normalizes after a PE
transpose back to natural layout.

ACT instruction stream is strictly Sin-phase, Exp-phase, Sin-phase so
only 3 activation-table loads are inserted.
"""

import sys, math

sys.path.insert(0, "/opt/trn_rl_repo")

import numpy as np
import concourse.bass as bass
import concourse.mybir as mybir
import concourse.dve_ops as dve_ops
from concourse.dve_ops import DveOp
from concourse.dve_spec import Spec, Src0, Src1, C0, C1, C2, lower as dve_lower
from concourse.dve_uop import DveOpSpec
from concourse.bacc import Bacc
from concourse.tile import TileContext
from concourse.bass_utils import run_bass_kernel_spmd
from contextlib import ExitStack

F32 = mybir.dt.float32
F32R = mybir.dt.float32r
BF16 = mybir.dt.bfloat16
AF = mybir.ActivationFunctionType
ALU = mybir.AluOpType

B, S, D, H = 2, 2048, 1024, 16
DH = 64
NH = 4            # heads per core
DC = NH * DH      # 256 feature columns per core
NB = S // 128     # 16 s-blocks
TWO_PI = 2.0 * math.pi
SQRT2 = math.sqrt(2.0)
INV_SQRT2 = 1.0 / SQRT2
EXP_SCALE = 1.0 / math.sqrt(2.0 * DH)
MAGIC = 12582912.0  # 1.5 * 2^23: (a + MAGIC) - MAGIC == round-to-nearest(a)
# out-stage chunks (block0, nblocks): small tail chunks shorten the
# last-exp -> last-DMA critical chain
OUT_CHUNKS = [(0, 4), (4, 4), (8, 4), (12, 2), (14, 2)]


# ---------------- custom DVE op: f = frac_rn(in0*s0 + s1) ----------------
def _frac_ref(in0, in1, s0, s1, imm2):
    a = np.float32(np.float32(in0 * np.float32(s0)) + np.float32(s1))
    t = np.float32(a + np.float32(imm2))
    u = np.float32(t - np.float32(imm2))
    return np.float32(a - u)


def _frac2_ref(in0, in1, s0, s1, imm2):
    a = np.float32(np.float32(in0 * in1) + np.float32(s0))
    t = np.float32(a + np.float32(s1))
    u = np.float32(t - np.float32(s1))
    return np.float32(a - u)


_fa = Src0 * C0 + C1
_FRAC_SPEC = Spec(body=_fa - ((_fa + C2) - C2), reference=_frac_ref)
_f2 = Src0 * Src1 + C0
_FRAC2_SPEC = Spec(body=_f2 - ((_f2 + C1) - C1), reference=_frac2_ref)


def _register_op(name, spec, rd1):
    for op in dve_ops.OPS:
        if op.name == name:
            return op
    row = max(dve_ops._SUB_OPCODE_FOR_NAME.values()) + 1
    assert row < 0x20
    dve_ops._SUB_OPCODE_FOR_NAME[name] = row
    shas = {}
    for ver in ("v3", "v4"):
        spec_c = DveOpSpec(name=name, opcode=row,
                           uops=dve_lower(spec, ver=ver), rd1_en=rd1)
        shas[ver] = spec_c.sha(ver)
    op = DveOp(name, spec, subdim=False, uops_sha=shas)
    dve_ops.OPS.append(op)
    dve_ops.CUSTOM_DVE_SPECS[name] = spec
    return op


FRAC = _register_op("FRAC_AFFINE_ANT", _FRAC_SPEC, False)
FRAC2 = _register_op("FRAC_MUL_ANT", _FRAC2_SPEC, True)


def _dup_mid(ap2d, n):
    """[128, F] AP -> [128, n, F] with stride-0 middle dim."""
    return bass.AP(tensor=ap2d.tensor, offset=ap2d.offset,
                   ap=[ap2d.ap[0], [0, n], ap2d.ap[-1]])


def _bcast_inner(ap2d, n):
    """[128, F] AP -> [128, F, n] with stride-0 inner dim."""
    return bass.AP(tensor=ap2d.tensor, offset=ap2d.offset,
                   ap=[ap2d.ap[0], ap2d.ap[-1], [0, n]])


def _build_packs(qc):
    """PSUM pack layout for one 512-wide q chunk: list of packs, each a list
    of (kb, qs, N, off) strips placed in a [128,1024] (2-bank) psum tile."""
    order = list(range(4 * qc)) + [4 * qc, 4 * qc + 1, 4 * qc + 3, 4 * qc + 2]
    packs, cur, off = [], [], 0
    for kb in order:
        if kb < 4 * qc:
            qs, N = 512 * qc, 512
        else:
            jj = kb - 4 * qc
            qs, N = 512 * qc + 128 * jj, 512 - 128 * jj
        o = off
        if o % 512 + N > 512:
            o = (o // 512 + 1) * 512
        if o + N > 1024:
            packs.append(cur)
            cur, o = [], 0
        cur.append((kb, qs, N, o))
        off = o + N
    if cur:
        packs.append(cur)
    return packs


def build_nc(c_v=0.125, c_o=0.125):
    """c_v / c_o: host-folded (b/2pi + 0.125) constants."""
    nc = Bacc(trn_type="TRN2")
    xin = nc.dram_tensor("xin", [S, DC], F32, kind="ExternalInput")
    sqk_d = nc.dram_tensor("sqk", [128, NH, 2], F32, kind="ExternalInput")
    phc_d = nc.dram_tensor("phc", [3, NH, 2, 128], F32, kind="ExternalInput")
    vp_d = nc.dram_tensor("vp", [128, DC], F32, kind="ExternalInput")
    op_d = nc.dram_tensor("opar", [128, DC], F32, kind="ExternalInput")
    out_d = nc.dram_tensor("out", [S, DC], F32, kind="ExternalOutput")
    ident_d = nc.inline_tensor(np.eye(128, dtype=np.float32), "ident")
    trow_np = np.stack([np.arange(S, dtype=np.float32),
                        np.arange(S, dtype=np.float32),
                        np.ones(S, dtype=np.float32)])
    trow_d = nc.inline_tensor(trow_np, "trow")

    with TileContext(nc) as tc, ExitStack() as ctx:
        sing = ctx.enter_context(tc.tile_pool(name="sing", bufs=1))
        qkpool = ctx.enter_context(tc.tile_pool(name="qkp", bufs=8))
        x2pool = ctx.enter_context(tc.tile_pool(name="x2p", bufs=3))
        mid = ctx.enter_context(tc.tile_pool(name="mid", bufs=5))
        otpool = ctx.enter_context(tc.tile_pool(name="otp", bufs=2))
        expool = ctx.enter_context(tc.tile_pool(name="exp", bufs=4))
        rvpool = ctx.enter_context(tc.tile_pool(name="rvp", bufs=2))
        svpool = ctx.enter_context(tc.tile_pool(name="svp", bufs=4))
        tiny = ctx.enter_context(tc.tile_pool(name="tiny", bufs=4))
        ropool = ctx.enter_context(tc.tile_pool(name="rop", bufs=3))
        pprep = ctx.enter_context(tc.tile_pool(name="pprep", bufs=2, space="PSUM"))
        psp = ctx.enter_context(tc.tile_pool(name="psp", bufs=2, space="PSUM"))
        pso = ctx.enter_context(tc.tile_pool(name="pso", bufs=1, space="PSUM"))
        psn = ctx.enter_context(tc.tile_pool(name="psn", bufs=1, space="PSUM"))

        # ---- two parallel HWDGE queues: x quarters 0-2 on the ACT queue,
        # params + x quarter 3 on the SP queue.
        x_s = sing.tile([128, NB, DC], F32)
        xin_r = xin[:, :].rearrange("(n p) d -> p n d", p=128)
        for qq in range(3):
            nc.scalar.dma_start(out=x_s[:, 4 * qq:4 * qq + 4, :],
                                in_=xin_r[:, 4 * qq:4 * qq + 4, :])
        vp = sing.tile([128, DC], F32)
        nc.sync.dma_start(out=vp, in_=vp_d[:, :])
        sqk = sing.tile([128, NH, 2], F32)
        nc.sync.dma_start(out=sqk, in_=sqk_d[:, :, :])
        ident = sing.tile([128, 128], F32)
        nc.sync.dma_start(out=ident, in_=ident_d[:, :])
        phcr = sing.tile([3, NH, 2, 128], F32R)
        nc.sync.dma_start(out=phcr, in_=phc_d[:, :, :, :].bitcast(F32R))
        trowr = sing.tile([3, S], F32R)
        nc.sync.dma_start(out=trowr, in_=trow_d[:, :].bitcast(F32R))
        nc.sync.dma_start(out=x_s[:, 12:16, :], in_=xin_r[:, 12:16, :])
        opr = sing.tile([128, DC], F32)
        nc.sync.dma_start(out=opr, in_=op_d[:, :])

        bz = sing.tile([128, 1], F32)
        nc.vector.memset(bz, 0.0)
        onat = sing.tile([128, NB, DC], F32)
        vaug = []
        for j in range(NH):
            t = sing.tile([128, NB, DH + 1], BF16, tag=f"vaug{j}")
            nc.vector.memset(t[:, :, DH:DH + 1], INV_SQRT2)
            vaug.append(t)



        QT, KT = [None] * NH, [None] * NH
        # ACT-stream phase tracking: the Tile scheduler reorders freely and
        # does not model activation-table loads; explicit deps pin a
        # 5-phase order (sins v+h0, exps h0, sins h1-3, exps h1-3, out sins)
        # so head 0's exps start as soon as its Q/K are ready.
        sins, exps, osins = [], [], []

        # ---------------- V (4 quarters) ----------------
        sv_tiles = []

        def v_quarter(qq):
            xh = x_s[:, 4 * qq:4 * qq + 4, :]
            mv = rvpool.tile([128, 4, DC], F32, tag="rv")
            nc.vector._custom_dve(FRAC2, out=mv, in0=xh,
                                  in1=_dup_mid(vp[:, :], 4), s0=c_v, s1=MAGIC)
            sv = svpool.tile([128, 4, DC], BF16, tag="sv")
            sins.append(nc.scalar.activation(out=sv, in_=mv, func=AF.Sin,
                                             bias=bz[:, 0:1], scale=TWO_PI))
            sv_tiles.append((qq, sv))

        def v_scatter(qq, sv):
            for j in range(NH):
                nc.vector.tensor_copy(out=vaug[j][:, 4 * qq:4 * qq + 4, 0:DH],
                                      in_=sv[:, :, DH * j:DH * j + DH])

        # ---------------- QK prep ----------------
        def x2_dup(j, quarters=(slice(0, NB),)):
            """x2[:, n, 0:64] = x2[:, n, 64:128] = x_s[:, n, 64j:64j+64]."""
            x2 = x2pool.tile([128, NB, 128], F32, tag="x2")
            for qs in quarters:
                for half in range(2):
                    nc.gpsimd.tensor_copy(
                        out=x2[:, qs, 64 * half:64 * half + 64],
                        in_=x_s[:, qs, DH * j:DH * j + DH])
            return x2

        def qk_prep(j, x2, mid_cb=None, percc=False):
            """Per cc: rank-2 q phase, then transposes of the dup'd x block;
            FRAC + Sin per proj, with a rank-2 (k - q) delta matmul
            retargeting the PSUM to k. percc interleaves the k half per cc;
            otherwise the whole q projection finishes first (earlier q sin).
            mid_cb emits between the halves (DVE-stream ordering control)."""
            mq = mid.tile([128, S], F32, tag="mid")
            mk = mid.tile([128, S], F32, tag="mid")
            tiles = []
            for cc in range(4):
                xq = pprep.tile([128, 512], F32, tag="prep")
                sl = slice(512 * cc, 512 * cc + 512)
                nc.tensor.matmul(xq, phcr[:, j, 0, :], trowr[:, sl],
                                 start=True, stop=True)
                for sb in range(4):
                    n = 4 * cc + sb
                    nc.tensor.matmul(xq[:, 128 * sb:128 * sb + 128],
                                     x2[:, n, :], ident,
                                     is_transpose=True, start=False, stop=True,
                                     skip_group_check=True)
                nc.vector._custom_dve(FRAC, out=mq[:, sl], in0=xq,
                                      s0=sqk[:, j, 0:1], s1=0.0, imm2=MAGIC)
                if percc:
                    nc.tensor.matmul(xq, phcr[:, j, 1, :], trowr[:, sl],
                                     start=False, stop=True,
                                     skip_group_check=True)
                    nc.vector._custom_dve(FRAC, out=mk[:, sl], in0=xq,
                                          s0=sqk[:, j, 1:2], s1=0.0, imm2=MAGIC)
                    if cc == 1 and mid_cb is not None:
                        mid_cb()
                else:
                    tiles.append((xq, sl))
            if not percc:
                if mid_cb is not None:
                    mid_cb()
                for (xq, sl) in tiles:
                    nc.tensor.matmul(xq, phcr[:, j, 1, :], trowr[:, sl],
                                     start=False, stop=True,
                                     skip_group_check=True)
                    nc.vector._custom_dve(FRAC, out=mk[:, sl], in0=xq,
                                          s0=sqk[:, j, 1:2], s1=0.0, imm2=MAGIC)
            tq = qkpool.tile([128, S], BF16, tag="qk")
            sins.append(nc.scalar.activation(out=tq, in_=mq, func=AF.Sin,
                                             bias=bz[:, 0:1], scale=TWO_PI))
            QT[j] = tq
            tk = qkpool.tile([128, S], BF16, tag="qk")
            sins.append(nc.scalar.activation(out=tk, in_=mk, func=AF.Sin,
                                             bias=bz[:, 0:1], scale=TWO_PI))
            KT[j] = tk

        # ---------------- attention ----------------
        def attention(j, post_qc=None):
            for qc in range(4):
                ot_ps = pso.tile([65, 512], F32, tag="po")
                packs = _build_packs(qc)
                n_av = 4 * qc + 4
                avi = 0
                for pack in packs:
                    sc = psp.tile([128, 1024], F32, tag="ps")
                    for (kb, qs, N, off) in pack:
                        nc.tensor.matmul(sc[:, off:off + N],
                                         KT[j][:, 128 * kb:128 * kb + 128],
                                         QT[j][:, qs:qs + N],
                                         start=True, stop=True)
                    width = pack[-1][3] + pack[-1][2]
                    ext = expool.tile([128, 1024], BF16, tag="ex")
                    exps.append(nc.scalar.activation(
                        out=ext[:, 0:width], in_=sc[:, 0:width],
                        func=AF.Exp, bias=bz[:, 0:1], scale=EXP_SCALE))
                    for (kb, qs, N, off) in pack:
                        if kb >= 4 * qc:  # diagonal strip: zero exp where q < k
                            nc.gpsimd.affine_select(
                                out=ext[:, off:off + 128], in_=ext[:, off:off + 128],
                                pattern=[[1, 128]], compare_op=ALU.is_ge, fill=0.0,
                                base=0, channel_multiplier=-1)
                    for (kb, qs, N, off) in pack:
                        q0 = qs - 512 * qc
                        nc.tensor.matmul(ot_ps[:, q0:q0 + N],
                                         vaug[j][:, kb, :],
                                         ext[:, off:off + N],
                                         start=(avi == 0), stop=(avi == n_av - 1))
                        avi += 1
                ot_s = otpool.tile([65, 512], F32, tag="ot")
                nc.vector.tensor_copy(out=ot_s, in_=ot_ps)
                on_ps = psn.tile([128, 4, DH + 1], F32, tag="pn")
                for t4 in range(4):
                    nc.tensor.matmul(on_ps[:, t4, :],
                                     ot_s[:, 128 * t4:128 * t4 + 128],
                                     ident[0:65, 0:65],
                                     is_transpose=True, start=True, stop=True)
                den = tiny.tile([128, 4], F32, tag="tiny")
                nc.vector.reciprocal(out=den, in_=on_ps[:, :, DH:DH + 1])
                nc.vector.tensor_tensor(
                    out=onat[:, 4 * qc:4 * qc + 4, DH * j:DH * j + DH],
                    in0=on_ps[:, :, 0:DH], in1=_bcast_inner(den, DH),
                    op=ALU.mult)
                if post_qc is not None:
                    post_qc(qc)

        # ---------------- emission order ----------------
        # Pool: head-0 x2 copies per quarter as x lands, then later heads'.
        # DVE: v FRAC2s first (x-gated), then head-0 FRACs.
        v_quarter(0)
        v_quarter(1)
        v_quarter(2)
        x2_0 = x2_dup(0, quarters=[slice(4 * q, 4 * q + 4) for q in range(4)])
        qk_prep(0, x2_0, mid_cb=lambda: v_quarter(3), percc=False)
        qk_prep(1, x2_dup(1), percc=True)
        for qq, sv in sv_tiles:
            v_scatter(qq, sv)
        # x2 copies for heads 2/3 early in the Pool stream (before att0's
        # data-gated affines); their preps' PE/DVE work interleaves with
        # attention(0) via per-qc callbacks
        x2_2 = x2_dup(2)
        x2_3 = x2_dup(3)

        def prep_post_qc(qc):
            if qc == 0:
                qk_prep(2, x2_2, percc=True)
            elif qc == 2:
                qk_prep(3, x2_3, percc=True)

        attention(0, post_qc=prep_post_qc)
        nsin_a = 6            # v + head-0 sins: phase A
        nexp_h0 = len(exps)
        attention(1)
        attention(2)

        # out-stage FRAC2s interleave with attention(3): chunk data for qc
        # is complete right after head 3's qc normalize
        out_r = out_d[:, :].rearrange("(n p) d -> p n d", p=128)

        def emit_chunk(ci):
            b0, nb = OUT_CHUNKS[ci]
            rv = ropool.tile([128, nb, DC], F32, tag=f"ro{nb}")
            nc.vector._custom_dve(FRAC2, out=rv, in0=onat[:, b0:b0 + nb, :],
                                  in1=_dup_mid(opr[:, :], nb), s0=c_o, s1=MAGIC)
            osins.append(nc.scalar.activation(out=rv, in_=rv, func=AF.Sin,
                                              bias=bz[:, 0:1], scale=TWO_PI))
            nc.vector.tensor_scalar(out=rv, in0=rv, scalar1=SQRT2,
                                    scalar2=None, op0=ALU.mult)
            nc.sync.dma_start(out=out_r[:, b0:b0 + nb, :], in_=rv)

        def out_post_qc(qc):
            if qc < 3:
                emit_chunk(qc)
            else:
                emit_chunk(3)
                emit_chunk(4)

        attention(3, post_qc=out_post_qc)

        # phase-order bridges (A < B < C < D < E):
        #   A: v + h0 sins, B: h0 exps, C: h1-3 sins, D: h1-3 exps, E: out sins
        def bridge(prev_phase, next_phase):
            first = next_phase[0]
            for p_i in prev_phase:
                bass._add_dep_helper(first.ins, p_i.ins, sync=True,
                                     reason="act-table-order")
            for n_i in next_phase[1:]:
                bass._add_dep_helper(n_i.ins, first.ins, sync=True,
                                     reason="act-table-order")

        sins_a, sins_c = sins[:nsin_a], sins[nsin_a:]
        exps_b, exps_d = exps[:nexp_h0], exps[nexp_h0:]
        bridge(sins_a, exps_b)
        bridge(exps_b, sins_c)
        bridge(sins_c, exps_d)
        bridge(exps_d, osins)

    nc.finalize()
    return nc


def _host_params(inputs, c):
    """Per-core input dict for core c."""
    b, g = c // 4, c % 4
    inv2pi = 1.0 / (2.0 * np.pi)
    x = np.asarray(inputs["x"], dtype=np.float32)
    xin = np.ascontiguousarray(x[b, :, DC * g:DC * g + DC])

    def f64(a):
        return np.asarray(a, dtype=np.float64)

    def hi_lo(v):
        """Split f64 v into f32 hi (12 mantissa bits, so hi*t is exact in
        f32r for t < 2^12) + f32 lo remainder."""
        m, e = np.frexp(v)
        hi = np.ldexp(np.round(m * 2**12) / 2**12, e).astype(np.float32)
        lo = (v - hi.astype(np.float64)).astype(np.float32)
        return hi, lo

    rows = np.arange(128) % DH
    cos_row = (np.arange(128) < DH).astype(np.float64) * 0.25
    sqk = np.zeros((128, NH, 2), dtype=np.float32)
    phc = np.zeros((3, NH, 2, 128), dtype=np.float32)
    for j in range(NH):
        h = NH * g + j
        ph_s = {}
        for pi, (wn, bn, pn) in enumerate([("w_q", "b_q", "phi_q"),
                                           ("w_k", "b_k", "phi_k")]):
            w = f64(inputs[wn])[h]
            bb = f64(inputs[bn])[h]
            ph = f64(inputs[pn])[h]
            s = (inv2pi / (1.0 + np.abs(w)))[rows]
            phi2 = (ph * inv2pi)[rows] / s
            c2 = ((bb * inv2pi)[rows] + cos_row) / s
            sqk[:, j, pi] = s
            ph_s[pi] = (phi2, c2)
        phq_hi, phq_lo = hi_lo(ph_s[0][0])
        phk_hi, _ = hi_lo(ph_s[1][0])
        dphi_hi = (phk_hi - phq_hi).astype(np.float64)
        dphi_lo = (ph_s[1][0] - ph_s[0][0] - dphi_hi).astype(np.float32)
        phc[0, j, 0, :] = phq_hi
        phc[1, j, 0, :] = phq_lo
        phc[2, j, 0, :] = ph_s[0][1]
        phc[0, j, 1, :] = dphi_hi                   # k - q phase delta
        phc[1, j, 1, :] = dphi_lo
        phc[2, j, 1, :] = ph_s[1][1] - ph_s[0][1]

    vp = np.zeros((128, DC), dtype=np.float32)
    wv = f64(inputs["w_v"])[NH * g:NH * g + NH].reshape(-1)
    vp[:, :] = (inv2pi / (1.0 + np.abs(wv)))[None, :]

    op = np.zeros((128, DC), dtype=np.float32)
    wo = f64(inputs["w_out"])[DC * g:DC * g + DC]
    op[:, :] = (inv2pi / (1.0 + np.abs(wo)))[None, :]

    return {"xin": xin, "sqk": sqk, "phc": phc, "vp": vp, "opar": op}


_NC_CACHE = {}


def kernel(**inputs) -> np.ndarray:
    in_maps = [_host_params(inputs, c) for c in range(8)]
    inv2pi = 1.0 / (2.0 * np.pi)
    bv = np.asarray(inputs["b_v"], dtype=np.float64).reshape(-1)
    bo = np.asarray(inputs["b_out"], dtype=np.float64).reshape(-1)
    assert np.all(bv == bv[0]) and np.all(bo == bo[0]), "non-uniform b_v/b_out unsupported"
    c_v = float(np.float32(bv[0] * inv2pi + 0.125))
    c_o = float(np.float32(bo[0] * inv2pi + 0.125))
    key = (c_v, c_o)
    if _NC_CACHE.get("key") != key:
        _NC_CACHE["nc"] = build_nc(c_v, c_o)
        _NC_CACHE["key"] = key
    nc = _NC_CACHE["nc"]
    res = run_bass_kernel_spmd(nc, in_maps, core_ids=list(range(8)))
    full = np.empty((B, S, D), dtype=np.float32)
    for c in range(8):
        b, g = c // 4, c % 4
        full[b, :, DC * g:DC * g + DC] = res.results[c]["out"]
    return full


# revision 93
# speedup vs baseline: 1.1632x; 1.0036x over previous
"""Trainium2 Bass kernel for nn_EulerFullAttention.

Math (per batch b, head h, dh=64):
  theta_q = x/(1+|w_q|) + b_q + t*phi_q ; Q = [cos(theta_q), sin(theta_q)]  (S,128)
  theta_k likewise ; K = [cos, sin]
  V = cos(theta_v)+sin(theta_v) = sqrt(2)*sin(theta_v + pi/4)              (S,64)
  scores = Q @ K^T / sqrt(128), causal softmax, out = attn @ V
  result = cos(theta_o)+sin(theta_o) = sqrt(2)*sin(theta_o + pi/4)

Distribution: 8 cores = 2 batches x 4 head-groups (4 heads each). Each core
computes its x[:, 256-col] slice end to end; no collectives.

Trig via a custom DVE op FRAC_AFFINE_ANT: f = a - ((a + C) - C) with
C = 1.5*2^23 rounds a = in*s + c to nearest in one instruction, so
sin(theta) = Sin(2*pi*f) with f in [-0.5, 0.5]. Cos rows get +0.25 in c.

qk prep: x's 64 head-features are duplicated into both partition halves
(x2, gpsimd copies), PE-transposed per 128-block into PSUM, and a rank-2
f32r matmul [phi'; c'] @ [t; 1] accumulates the t*phi_q/(2pi*s)+c phase
(divided by the x scale s on host). One FRAC(in*s) + Sin(bf16 out) per
projection; the k projection reuses the same PSUM via a second rank-2
matmul adding the (k - q) phase delta.

Attention in transposed layout: scoresT[k, q] = KT.T @ QT, all-bf16
matmuls (1 cyc/row at any width). exp via ACT from PSUM -> bf16 SBUF;
causal via block structure + affine_select on diagonal blocks. attn@V
accumulates outT[65, 512] per 512-wide q chunk with lhsT = [V/sqrt2*...
actually [sv | 1/sqrt2] where sv = V/sqrt2, so row 64 is rowsum/sqrt2 and
one tensor_tensor divide per (head, chunk) normalizes after a PE
transpose back to natural layout.

ACT instruction stream is strictly Sin-phase, Exp-phase, Sin-phase so
only 3 activation-table loads are inserted.
"""

import sys, math

sys.path.insert(0, "/opt/trn_rl_repo")

import numpy as np
import concourse.bass as bass
import concourse.mybir as mybir
import concourse.dve_ops as dve_ops
from concourse.dve_ops import DveOp
from concourse.dve_spec import Spec, Src0, Src1, C0, C1, C2, lower as dve_lower
from concourse.dve_uop import DveOpSpec
from concourse.bacc import Bacc
from concourse.tile import TileContext
from concourse.bass_utils import run_bass_kernel_spmd
from contextlib import ExitStack

F32 = mybir.dt.float32
F32R = mybir.dt.float32r
BF16 = mybir.dt.bfloat16
AF = mybir.ActivationFunctionType
ALU = mybir.AluOpType

B, S, D, H = 2, 2048, 1024, 16
DH = 64
NH = 4            # heads per core
DC = NH * DH      # 256 feature columns per core
NB = S // 128     # 16 s-blocks
TWO_PI = 2.0 * math.pi
SQRT2 = math.sqrt(2.0)
INV_SQRT2 = 1.0 / SQRT2
EXP_SCALE = 1.0 / math.sqrt(2.0 * DH)
MAGIC = 12582912.0  # 1.5 * 2^23: (a + MAGIC) - MAGIC == round-to-nearest(a)
# out-stage chunks (block0, nblocks): small tail chunks shorten the
# last-exp -> last-DMA critical chain
OUT_CHUNKS = [(0, 4), (4, 4), (8, 4), (12, 2), (14, 2)]


# ---------------- custom DVE op: f = frac_rn(in0*s0 + s1) ----------------
def _frac_ref(in0, in1, s0, s1, imm2):
    a = np.float32(np.float32(in0 * np.float32(s0)) + np.float32(s1))
    t = np.float32(a + np.float32(imm2))
    u = np.float32(t - np.float32(imm2))
    return np.float32(a - u)


def _frac2_ref(in0, in1, s0, s1, imm2):
    a = np.float32(np.float32(in0 * in1) + np.float32(s0))
    t = np.float32(a + np.float32(s1))
    u = np.float32(t - np.float32(s1))
    return np.float32(a - u)


_fa = Src0 * C0 + C1
_FRAC_SPEC = Spec(body=_fa - ((_fa + C2) - C2), reference=_frac_ref)
_f2 = Src0 * Src1 + C0
_FRAC2_SPEC = Spec(body=_f2 - ((_f2 + C1) - C1), reference=_frac2_ref)


def _register_op(name, spec, rd1):
    for op in dve_ops.OPS:
        if op.name == name:
            return op
    row = max(dve_ops._SUB_OPCODE_FOR_NAME.values()) + 1
    assert row < 0x20
    dve_ops._SUB_OPCODE_FOR_NAME[name] = row
    shas = {}
    for ver in ("v3", "v4"):
        spec_c = DveOpSpec(name=name, opcode=row,
                           uops=dve_lower(spec, ver=ver), rd1_en=rd1)
        shas[ver] = spec_c.sha(ver)
    op = DveOp(name, spec, subdim=False, uops_sha=shas)
    dve_ops.OPS.append(op)
    dve_ops.CUSTOM_DVE_SPECS[name] = spec
    return op


FRAC = _register_op("FRAC_AFFINE_ANT", _FRAC_SPEC, False)
FRAC2 = _register_op("FRAC_MUL_ANT", _FRAC2_SPEC, True)


def _dup_mid(ap2d, n):
    """[128, F] AP -> [128, n, F] with stride-0 middle dim."""
    return bass.AP(tensor=ap2d.tensor, offset=ap2d.offset,
                   ap=[ap2d.ap[0], [0, n], ap2d.ap[-1]])


def _bcast_inner(ap2d, n):
    """[128, F] AP -> [128, F, n] with stride-0 inner dim."""
    return bass.AP(tensor=ap2d.tensor, offset=ap2d.offset,
                   ap=[ap2d.ap[0], ap2d.ap[-1], [0, n]])


def _build_packs(qc):
    """PSUM pack layout for one 512-wide q chunk: list of packs, each a list
    of (kb, qs, N, off) strips placed in a [128,1024] (2-bank) psum tile."""
    order = list(range(4 * qc)) + [4 * qc, 4 * qc + 1, 4 * qc + 3, 4 * qc + 2]
    packs, cur, off = [], [], 0
    for kb in order:
        if kb < 4 * qc:
            qs, N = 512 * qc, 512
        else:
            jj = kb - 4 * qc
            qs, N = 512 * qc + 128 * jj, 512 - 128 * jj
        o = off
        if o % 512 + N > 512:
            o = (o // 512 + 1) * 512
        if o + N > 1024:
            packs.append(cur)
            cur, o = [], 0
        cur.append((kb, qs, N, o))
        off = o + N
    if cur:
        packs.append(cur)
    return packs


def build_nc(c_v=0.125, c_o=0.125):
    """c_v / c_o: host-folded (b/2pi + 0.125) constants."""
    nc = Bacc(trn_type="TRN2")
    xin = nc.dram_tensor("xin", [S, DC], F32, kind="ExternalInput")
    sqk_d = nc.dram_tensor("sqk", [128, NH, 2], F32, kind="ExternalInput")
    phc_d = nc.dram_tensor("phc", [3, NH, 2, 128], F32, kind="ExternalInput")
    vp_d = nc.dram_tensor("vp", [128, DC], F32, kind="ExternalInput")
    op_d = nc.dram_tensor("opar", [128, DC], F32, kind="ExternalInput")
    out_d = nc.dram_tensor("out", [S, DC], F32, kind="ExternalOutput")
    ident_d = nc.inline_tensor(np.eye(128, dtype=np.float32), "ident")
    trow_np = np.stack([np.arange(S, dtype=np.float32),
                        np.arange(S, dtype=np.float32),
                        np.ones(S, dtype=np.float32)])
    trow_d = nc.inline_tensor(trow_np, "trow")

    with TileContext(nc) as tc, ExitStack() as ctx:
        sing = ctx.enter_context(tc.tile_pool(name="sing", bufs=1))
        qkpool = ctx.enter_context(tc.tile_pool(name="qkp", bufs=8))
        x2pool = ctx.enter_context(tc.tile_pool(name="x2p", bufs=3))
        mid = ctx.enter_context(tc.tile_pool(name="mid", bufs=5))
        otpool = ctx.enter_context(tc.tile_pool(name="otp", bufs=2))
        expool = ctx.enter_context(tc.tile_pool(name="exp", bufs=8))
        rvpool = ctx.enter_context(tc.tile_pool(name="rvp", bufs=2))
        svpool = ctx.enter_context(tc.tile_pool(name="svp", bufs=4))
        tiny = ctx.enter_context(tc.tile_pool(name="tiny", bufs=4))
        ropool = ctx.enter_context(tc.tile_pool(name="rop", bufs=3))
        pprep = ctx.enter_context(tc.tile_pool(name="pprep", bufs=2, space="PSUM"))
        psp = ctx.enter_context(tc.tile_pool(name="psp", bufs=2, space="PSUM"))
        pso = ctx.enter_context(tc.tile_pool(name="pso", bufs=1, space="PSUM"))
        psn = ctx.enter_context(tc.tile_pool(name="psn", bufs=1, space="PSUM"))

        # ---- two parallel HWDGE queues: x quarters 0-2 on the ACT queue,
        # params + x quarter 3 on the SP queue.
        x_s = sing.tile([128, NB, DC], F32)
        xin_r = xin[:, :].rearrange("(n p) d -> p n d", p=128)
        for qq in range(3):
            nc.scalar.dma_start(out=x_s[:, 4 * qq:4 * qq + 4, :],
                                in_=xin_r[:, 4 * qq:4 * qq + 4, :])
        vp = sing.tile([128, DC], F32)
        nc.sync.dma_start(out=vp, in_=vp_d[:, :])
        sqk = sing.tile([128, NH, 2], F32)
        nc.sync.dma_start(out=sqk, in_=sqk_d[:, :, :])
        ident = sing.tile([128, 128], F32)
        nc.sync.dma_start(out=ident, in_=ident_d[:, :])
        phcr = sing.tile([3, NH, 2, 128], F32R)
        nc.sync.dma_start(out=phcr, in_=phc_d[:, :, :, :].bitcast(F32R))
        trowr = sing.tile([3, S], F32R)
        nc.sync.dma_start(out=trowr, in_=trow_d[:, :].bitcast(F32R))
        nc.sync.dma_start(out=x_s[:, 12:16, :], in_=xin_r[:, 12:16, :])
        opr = sing.tile([128, DC], F32)
        nc.sync.dma_start(out=opr, in_=op_d[:, :])

        bz = sing.tile([128, 1], F32)
        nc.vector.memset(bz, 0.0)
        onat = sing.tile([128, NB, DC], F32)
        vaug = []
        for j in range(NH):
            t = sing.tile([128, NB, DH + 1], BF16, tag=f"vaug{j}")
            nc.vector.memset(t[:, :, DH:DH + 1], INV_SQRT2)
            vaug.append(t)



        QT, KT = [None] * NH, [None] * NH
        # ACT-stream phase tracking: the Tile scheduler reorders freely and
        # does not model activation-table loads; explicit deps pin a
        # 5-phase order (sins v+h0, exps h0, sins h1-3, exps h1-3, out sins)
        # so head 0's exps start as soon as its Q/K are ready.
        sins, exps, osins = [], [], []

        # ---------------- V (4 quarters) ----------------
        sv_tiles = []

        def v_quarter(qq):
            xh = x_s[:, 4 * qq:4 * qq + 4, :]
            mv = rvpool.tile([128, 4, DC], F32, tag="rv")
            nc.vector._custom_dve(FRAC2, out=mv, in0=xh,
                                  in1=_dup_mid(vp[:, :], 4), s0=c_v, s1=MAGIC)
            sv = svpool.tile([128, 4, DC], BF16, tag="sv")
            sins.append(nc.scalar.activation(out=sv, in_=mv, func=AF.Sin,
                                             bias=bz[:, 0:1], scale=TWO_PI))
            sv_tiles.append((qq, sv))

        def v_scatter(qq, sv):
            for j in range(NH):
                nc.vector.tensor_copy(out=vaug[j][:, 4 * qq:4 * qq + 4, 0:DH],
                                      in_=sv[:, :, DH * j:DH * j + DH])

        # ---------------- QK prep ----------------
        def x2_dup(j, quarters=(slice(0, NB),)):
            """x2[:, n, 0:64] = x2[:, n, 64:128] = x_s[:, n, 64j:64j+64]."""
            x2 = x2pool.tile([128, NB, 128], F32, tag="x2")
            for qs in quarters:
                for half in range(2):
                    nc.gpsimd.tensor_copy(
                        out=x2[:, qs, 64 * half:64 * half + 64],
                        in_=x_s[:, qs, DH * j:DH * j + DH])
            return x2

        def qk_prep(j, x2, mid_cb=None, percc=False):
            """Per cc: rank-2 q phase, then transposes of the dup'd x block;
            FRAC + Sin per proj, with a rank-2 (k - q) delta matmul
            retargeting the PSUM to k. percc interleaves the k half per cc;
            otherwise the whole q projection finishes first (earlier q sin).
            mid_cb emits between the halves (DVE-stream ordering control)."""
            mq = mid.tile([128, S], F32, tag="mid")
            mk = mid.tile([128, S], F32, tag="mid")
            tiles = []
            for cc in range(4):
                xq = pprep.tile([128, 512], F32, tag="prep")
                sl = slice(512 * cc, 512 * cc + 512)
                nc.tensor.matmul(xq, phcr[:, j, 0, :], trowr[:, sl],
                                 start=True, stop=True)
                for sb in range(4):
                    n = 4 * cc + sb
                    nc.tensor.matmul(xq[:, 128 * sb:128 * sb + 128],
                                     x2[:, n, :], ident,
                                     is_transpose=True, start=False, stop=True,
                                     skip_group_check=True)
                nc.vector._custom_dve(FRAC, out=mq[:, sl], in0=xq,
                                      s0=sqk[:, j, 0:1], s1=0.0, imm2=MAGIC)
                if percc:
                    nc.tensor.matmul(xq, phcr[:, j, 1, :], trowr[:, sl],
                                     start=False, stop=True,
                                     skip_group_check=True)
                    nc.vector._custom_dve(FRAC, out=mk[:, sl], in0=xq,
                                          s0=sqk[:, j, 1:2], s1=0.0, imm2=MAGIC)
                    if cc == 1 and mid_cb is not None:
                        mid_cb()
                else:
                    tiles.append((xq, sl))
            if not percc:
                if mid_cb is not None:
                    mid_cb()
                for (xq, sl) in tiles:
                    nc.tensor.matmul(xq, phcr[:, j, 1, :], trowr[:, sl],
                                     start=False, stop=True,
                                     skip_group_check=True)
                    nc.vector._custom_dve(FRAC, out=mk[:, sl], in0=xq,
                                          s0=sqk[:, j, 1:2], s1=0.0, imm2=MAGIC)
            tq = qkpool.tile([128, S], BF16, tag="qk")
            sins.append(nc.scalar.activation(out=tq, in_=mq, func=AF.Sin,
                                             bias=bz[:, 0:1], scale=TWO_PI))
            QT[j] = tq
            tk = qkpool.tile([128, S], BF16, tag="qk")
            sins.append(nc.scalar.activation(out=tk, in_=mk, func=AF.Sin,
                                             bias=bz[:, 0:1], scale=TWO_PI))
            KT[j] = tk

        # ---------------- attention ----------------
        def attention(j, post_qc=None):
            for qc in range(4):
                ot_ps = pso.tile([65, 512], F32, tag="po")
                packs = _build_packs(qc)
                n_av = 4 * qc + 4
                avi = 0
                for pack in packs:
                    sc = psp.tile([128, 1024], F32, tag="ps")
                    for (kb, qs, N, off) in pack:
                        nc.tensor.matmul(sc[:, off:off + N],
                                         KT[j][:, 128 * kb:128 * kb + 128],
                                         QT[j][:, qs:qs + N],
                                         start=True, stop=True)
                    width = pack[-1][3] + pack[-1][2]
                    ext = expool.tile([128, 1024], BF16, tag="ex")
                    exps.append(nc.scalar.activation(
                        out=ext[:, 0:width], in_=sc[:, 0:width],
                        func=AF.Exp, bias=bz[:, 0:1], scale=EXP_SCALE))
                    for (kb, qs, N, off) in pack:
                        if kb >= 4 * qc:  # diagonal strip: zero exp where q < k
                            nc.gpsimd.affine_select(
                                out=ext[:, off:off + 128], in_=ext[:, off:off + 128],
                                pattern=[[1, 128]], compare_op=ALU.is_ge, fill=0.0,
                                base=0, channel_multiplier=-1)
                    for (kb, qs, N, off) in pack:
                        q0 = qs - 512 * qc
                        nc.tensor.matmul(ot_ps[:, q0:q0 + N],
                                         vaug[j][:, kb, :],
                                         ext[:, off:off + N],
                                         start=(avi == 0), stop=(avi == n_av - 1))
                        avi += 1
                ot_s = otpool.tile([65, 512], F32, tag="ot")
                nc.vector.tensor_copy(out=ot_s, in_=ot_ps)
                on_ps = psn.tile([128, 4, DH + 1], F32, tag="pn")
                for t4 in range(4):
                    nc.tensor.matmul(on_ps[:, t4, :],
                                     ot_s[:, 128 * t4:128 * t4 + 128],
                                     ident[0:65, 0:65],
                                     is_transpose=True, start=True, stop=True)
                den = tiny.tile([128, 4], F32, tag="tiny")
                nc.vector.reciprocal(out=den, in_=on_ps[:, :, DH:DH + 1])
                nc.vector.tensor_tensor(
                    out=onat[:, 4 * qc:4 * qc + 4, DH * j:DH * j + DH],
                    in0=on_ps[:, :, 0:DH], in1=_bcast_inner(den, DH),
                    op=ALU.mult)
                if post_qc is not None:
                    post_qc(qc)

        # ---------------- emission order ----------------
        # Pool: head-0 x2 copies per quarter as x lands, then later heads'.
        # DVE: v FRAC2s first (x-gated), then head-0 FRACs.
        v_quarter(0)
        v_quarter(1)
        v_quarter(2)
        x2_0 = x2_dup(0, quarters=[slice(4 * q, 4 * q + 4) for q in range(4)])
        qk_prep(0, x2_0, mid_cb=lambda: v_quarter(3), percc=False)
        qk_prep(1, x2_dup(1), percc=True)
        for qq, sv in sv_tiles:
            v_scatter(qq, sv)
        # x2 copies for heads 2/3 early in the Pool stream (before att0's
        # data-gated affines); their preps' PE/DVE work interleaves with
        # attention(0) via per-qc callbacks
        x2_2 = x2_dup(2)
        x2_3 = x2_dup(3)

        def prep_post_qc(qc):
            if qc == 0:
                qk_prep(2, x2_2, percc=True)
            elif qc == 2:
                qk_prep(3, x2_3, percc=True)

        attention(0, post_qc=prep_post_qc)
        nsin_a = 6            # v + head-0 sins: phase A
        nexp_h0 = len(exps)
        attention(1)
        attention(2)

        # out-stage FRAC2s interleave with attention(3): chunk data for qc
        # is complete right after head 3's qc normalize
        out_r = out_d[:, :].rearrange("(n p) d -> p n d", p=128)

        def emit_chunk(ci):
            b0, nb = OUT_CHUNKS[ci]
            rv = ropool.tile([128, nb, DC], F32, tag=f"ro{nb}")
            nc.vector._custom_dve(FRAC2, out=rv, in0=onat[:, b0:b0 + nb, :],
                                  in1=_dup_mid(opr[:, :], nb), s0=c_o, s1=MAGIC)
            osins.append(nc.scalar.activation(out=rv, in_=rv, func=AF.Sin,
                                              bias=bz[:, 0:1], scale=TWO_PI))
            nc.vector.tensor_scalar(out=rv, in0=rv, scalar1=SQRT2,
                                    scalar2=None, op0=ALU.mult)
            nc.sync.dma_start(out=out_r[:, b0:b0 + nb, :], in_=rv)

        def out_post_qc(qc):
            if qc < 3:
                emit_chunk(qc)
            else:
                emit_chunk(3)
                emit_chunk(4)

        attention(3, post_qc=out_post_qc)

        # phase-order bridges (A < B < C < D < E):
        #   A: v + h0 sins, B: h0 exps, C: h1-3 sins, D: h1-3 exps, E: out sins
        def bridge(prev_phase, next_phase):
            first = next_phase[0]
            for p_i in prev_phase:
                bass._add_dep_helper(first.ins, p_i.ins, sync=True,
                                     reason="act-table-order")
            for n_i in next_phase[1:]:
                bass._add_dep_helper(n_i.ins, first.ins, sync=True,
                                     reason="act-table-order")

        sins_a, sins_c = sins[:nsin_a], sins[nsin_a:]
        exps_b, exps_d = exps[:nexp_h0], exps[nexp_h0:]
        bridge(sins_a, exps_b)
        bridge(exps_b, sins_c)
        bridge(sins_c, exps_d)
        bridge(exps_d, osins)

    nc.finalize()
    return nc


def _host_params(inputs, c):
    """Per-core input dict for core c."""
    b, g = c // 4, c % 4
    inv2pi = 1.0 / (2.0 * np.pi)
    x = np.asarray(inputs["x"], dtype=np.float32)
    xin = np.ascontiguousarray(x[b, :, DC * g:DC * g + DC])

    def f64(a):
        return np.asarray(a, dtype=np.float64)

    def hi_lo(v):
        """Split f64 v into f32 hi (12 mantissa bits, so hi*t is exact in
        f32r for t < 2^12) + f32 lo remainder."""
        m, e = np.frexp(v)
        hi = np.ldexp(np.round(m * 2**12) / 2**12, e).astype(np.float32)
        lo = (v - hi.astype(np.float64)).astype(np.float32)
        return hi, lo

    rows = np.arange(128) % DH
    cos_row = (np.arange(128) < DH).astype(np.float64) * 0.25
    sqk = np.zeros((128, NH, 2), dtype=np.float32)
    phc = np.zeros((3, NH, 2, 128), dtype=np.float32)
    for j in range(NH):
        h = NH * g + j
        ph_s = {}
        for pi, (wn, bn, pn) in enumerate([("w_q", "b_q", "phi_q"),
                                           ("w_k", "b_k", "phi_k")]):
            w = f64(inputs[wn])[h]
            bb = f64(inputs[bn])[h]
            ph = f64(inputs[pn])[h]
            s = (inv2pi / (1.0 + np.abs(w)))[rows]
            phi2 = (ph * inv2pi)[rows] / s
            c2 = ((bb * inv2pi)[rows] + cos_row) / s
            sqk[:, j, pi] = s
            ph_s[pi] = (phi2, c2)
        phq_hi, phq_lo = hi_lo(ph_s[0][0])
        phk_hi, _ = hi_lo(ph_s[1][0])
        dphi_hi = (phk_hi - phq_hi).astype(np.float64)
        dphi_lo = (ph_s[1][0] - ph_s[0][0] - dphi_hi).astype(np.float32)
        phc[0, j, 0, :] = phq_hi
        phc[1, j, 0, :] = phq_lo
        phc[2, j, 0, :] = ph_s[0][1]
        phc[0, j, 1, :] = dphi_hi                   # k - q phase delta
        phc[1, j, 1, :] = dphi_lo
        phc[2, j, 1, :] = ph_s[1][1] - ph_s[0][1]

    vp = np.zeros((128, DC), dtype=np.float32)
    wv = f64(inputs["w_v"])[NH * g:NH * g + NH].reshape(-1)
    vp[:, :] = (inv2pi / (1.0 + np.abs(wv)))[None, :]

    op = np.zeros((128, DC), dtype=np.float32)
    wo = f64(inputs["w_out"])[DC * g:DC * g + DC]
    op[:, :] = (inv2pi / (1.0 + np.abs(wo)))[None, :]

    return {"xin": xin, "sqk": sqk, "phc": phc, "vp": vp, "opar": op}


_NC_CACHE = {}


def kernel(**inputs) -> np.ndarray:
    in_maps = [_host_params(inputs, c) for c in range(8)]
    inv2pi = 1.0 / (2.0 * np.pi)
    bv = np.asarray(inputs["b_v"], dtype=np.float64).reshape(-1)
    bo = np.asarray(inputs["b_out"], dtype=np.float64).reshape(-1)
    assert np.all(bv == bv[0]) and np.all(bo == bo[0]), "non-uniform b_v/b_out unsupported"
    c_v = float(np.float32(bv[0] * inv2pi + 0.125))
    c_o = float(np.float32(bo[0] * inv2pi + 0.125))
    key = (c_v, c_o)
    if _NC_CACHE.get("key") != key:
        _NC_CACHE["nc"] = build_nc(c_v, c_o)
        _NC_CACHE["key"] = key
    nc = _NC_CACHE["nc"]
    res = run_bass_kernel_spmd(nc, in_maps, core_ids=list(range(8)))
    full = np.empty((B, S, D), dtype=np.float32)
    for c in range(8):
        b, g = c // 4, c % 4
        full[b, :, DC * g:DC * g + DC] = res.results[c]["out"]
    return full


# revision 99
# speedup vs baseline: 1.1899x; 1.0229x over previous
"""Trainium2 Bass kernel for nn_EulerFullAttention.

Math (per batch b, head h, dh=64):
  theta_q = x/(1+|w_q|) + b_q + t*phi_q ; Q = [cos(theta_q), sin(theta_q)]  (S,128)
  theta_k likewise ; K = [cos, sin]
  V = cos(theta_v)+sin(theta_v) = sqrt(2)*sin(theta_v + pi/4)              (S,64)
  scores = Q @ K^T / sqrt(128), causal softmax, out = attn @ V
  result = cos(theta_o)+sin(theta_o) = sqrt(2)*sin(theta_o + pi/4)

Distribution: 8 cores = 2 batches x 4 head-groups (4 heads each). Each core
computes its x[:, 256-col] slice end to end; no collectives.

Trig via a custom DVE op FRAC_AFFINE_ANT: f = a - ((a + C) - C) with
C = 1.5*2^23 rounds a = in*s + c to nearest in one instruction, so
sin(theta) = Sin(2*pi*f) with f in [-0.5, 0.5]. Cos rows get +0.25 in c.

qk prep: x's 64 head-features are duplicated into both partition halves
(x2, gpsimd copies), PE-transposed per 128-block into PSUM, and a rank-2
f32r matmul [phi'; c'] @ [t; 1] accumulates the t*phi_q/(2pi*s)+c phase
(divided by the x scale s on host). One FRAC(in*s) + Sin(bf16 out) per
projection; the k projection reuses the same PSUM via a second rank-2
matmul adding the (k - q) phase delta.

Attention in transposed layout: scoresT[k, q] = KT.T @ QT, all-bf16
matmuls (1 cyc/row at any width). exp via ACT from PSUM -> bf16 SBUF;
causal via block structure + affine_select on diagonal blocks. attn@V
accumulates outT[65, 512] per 512-wide q chunk with lhsT = [V/sqrt2*...
actually [sv | 1/sqrt2] where sv = V/sqrt2, so row 64 is rowsum/sqrt2 and
one tensor_tensor divide per (head, chunk) normalizes after a PE
transpose back to natural layout.

ACT instruction stream is strictly Sin-phase, Exp-phase, Sin-phase so
only 3 activation-table loads are inserted.
"""

import sys, math

sys.path.insert(0, "/opt/trn_rl_repo")

import numpy as np
import concourse.bass as bass
import concourse.mybir as mybir
import concourse.dve_ops as dve_ops
from concourse.dve_ops import DveOp
from concourse.dve_spec import Spec, Src0, Src1, C0, C1, C2, lower as dve_lower
from concourse.dve_uop import DveOpSpec
from concourse.bacc import Bacc
from concourse.tile import TileContext
from concourse.bass_utils import run_bass_kernel_spmd
from contextlib import ExitStack

F32 = mybir.dt.float32
F32R = mybir.dt.float32r
BF16 = mybir.dt.bfloat16
AF = mybir.ActivationFunctionType
ALU = mybir.AluOpType

B, S, D, H = 2, 2048, 1024, 16
DH = 64
NH = 4            # heads per core
DC = NH * DH      # 256 feature columns per core
NB = S // 128     # 16 s-blocks
TWO_PI = 2.0 * math.pi
SQRT2 = math.sqrt(2.0)
INV_SQRT2 = 1.0 / SQRT2
EXP_SCALE = 1.0 / math.sqrt(2.0 * DH)
MAGIC = 12582912.0  # 1.5 * 2^23: (a + MAGIC) - MAGIC == round-to-nearest(a)
# out-stage chunks (block0, nblocks): small tail chunks shorten the
# last-exp -> last-DMA critical chain
OUT_CHUNKS = [(0, 4), (4, 4), (8, 4), (12, 2), (14, 2)]


# ---------------- custom DVE op: f = frac_rn(in0*s0 + s1) ----------------
def _frac_ref(in0, in1, s0, s1, imm2):
    a = np.float32(np.float32(in0 * np.float32(s0)) + np.float32(s1))
    t = np.float32(a + np.float32(imm2))
    u = np.float32(t - np.float32(imm2))
    return np.float32(a - u)


def _frac2_ref(in0, in1, s0, s1, imm2):
    a = np.float32(np.float32(in0 * in1) + np.float32(s0))
    t = np.float32(a + np.float32(s1))
    u = np.float32(t - np.float32(s1))
    return np.float32(a - u)


_fa = Src0 * C0 + C1
_FRAC_SPEC = Spec(body=_fa - ((_fa + C2) - C2), reference=_frac_ref)
_f2 = Src0 * Src1 + C0
_FRAC2_SPEC = Spec(body=_f2 - ((_f2 + C1) - C1), reference=_frac2_ref)


def _register_op(name, spec, rd1):
    for op in dve_ops.OPS:
        if op.name == name:
            return op
    row = max(dve_ops._SUB_OPCODE_FOR_NAME.values()) + 1
    assert row < 0x20
    dve_ops._SUB_OPCODE_FOR_NAME[name] = row
    shas = {}
    for ver in ("v3", "v4"):
        spec_c = DveOpSpec(name=name, opcode=row,
                           uops=dve_lower(spec, ver=ver), rd1_en=rd1)
        shas[ver] = spec_c.sha(ver)
    op = DveOp(name, spec, subdim=False, uops_sha=shas)
    dve_ops.OPS.append(op)
    dve_ops.CUSTOM_DVE_SPECS[name] = spec
    return op


FRAC = _register_op("FRAC_AFFINE_ANT", _FRAC_SPEC, False)
FRAC2 = _register_op("FRAC_MUL_ANT", _FRAC2_SPEC, True)


def _dup_mid(ap2d, n):
    """[128, F] AP -> [128, n, F] with stride-0 middle dim."""
    return bass.AP(tensor=ap2d.tensor, offset=ap2d.offset,
                   ap=[ap2d.ap[0], [0, n], ap2d.ap[-1]])


def _bcast_inner(ap2d, n):
    """[128, F] AP -> [128, F, n] with stride-0 inner dim."""
    return bass.AP(tensor=ap2d.tensor, offset=ap2d.offset,
                   ap=[ap2d.ap[0], ap2d.ap[-1], [0, n]])


def _build_packs(qc):
    """PSUM pack layout for one 512-wide q chunk: list of packs, each a list
    of (kb, qs, N, off) strips placed in a [128,1024] (2-bank) psum tile."""
    order = list(range(4 * qc)) + [4 * qc, 4 * qc + 1, 4 * qc + 3, 4 * qc + 2]
    packs, cur, off = [], [], 0
    for kb in order:
        if kb < 4 * qc:
            qs, N = 512 * qc, 512
        else:
            jj = kb - 4 * qc
            qs, N = 512 * qc + 128 * jj, 512 - 128 * jj
        o = off
        if o % 512 + N > 512:
            o = (o // 512 + 1) * 512
        if o + N > 1024:
            packs.append(cur)
            cur, o = [], 0
        cur.append((kb, qs, N, o))
        off = o + N
    if cur:
        packs.append(cur)
    return packs


def build_nc(c_v=0.125, c_o=0.125):
    """c_v / c_o: host-folded (b/2pi + 0.125) constants."""
    nc = Bacc(trn_type="TRN2")
    xin = nc.dram_tensor("xin", [S, DC], F32, kind="ExternalInput")
    sqk_d = nc.dram_tensor("sqk", [128, NH, 2], F32, kind="ExternalInput")
    phc_d = nc.dram_tensor("phc", [3, NH, 2, 128], F32, kind="ExternalInput")
    vp_d = nc.dram_tensor("vp", [128, DC], F32, kind="ExternalInput")
    op_d = nc.dram_tensor("opar", [128, DC], F32, kind="ExternalInput")
    out_d = nc.dram_tensor("out", [S, DC], F32, kind="ExternalOutput")
    ident_d = nc.inline_tensor(np.eye(128, dtype=np.float32), "ident")
    trow_np = np.stack([np.arange(S, dtype=np.float32),
                        np.arange(S, dtype=np.float32),
                        np.ones(S, dtype=np.float32)])
    trow_d = nc.inline_tensor(trow_np, "trow")

    with TileContext(nc) as tc, ExitStack() as ctx:
        sing = ctx.enter_context(tc.tile_pool(name="sing", bufs=1))
        qkpool = ctx.enter_context(tc.tile_pool(name="qkp", bufs=8))
        x2pool = ctx.enter_context(tc.tile_pool(name="x2p", bufs=3))
        mid = ctx.enter_context(tc.tile_pool(name="mid", bufs=5))
        otpool = ctx.enter_context(tc.tile_pool(name="otp", bufs=2))
        expool = ctx.enter_context(tc.tile_pool(name="exp", bufs=8))
        rvpool = ctx.enter_context(tc.tile_pool(name="rvp", bufs=2))
        svpool = ctx.enter_context(tc.tile_pool(name="svp", bufs=4))
        tiny = ctx.enter_context(tc.tile_pool(name="tiny", bufs=4))
        ropool = ctx.enter_context(tc.tile_pool(name="rop", bufs=3))
        pprep = ctx.enter_context(tc.tile_pool(name="pprep", bufs=2, space="PSUM"))
        psp = ctx.enter_context(tc.tile_pool(name="psp", bufs=2, space="PSUM"))
        pso = ctx.enter_context(tc.tile_pool(name="pso", bufs=1, space="PSUM"))
        psn = ctx.enter_context(tc.tile_pool(name="psn", bufs=1, space="PSUM"))

        # ---- two parallel HWDGE queues: x quarters 0-2 on the ACT queue,
        # params + x quarter 3 on the SP queue.
        x_s = sing.tile([128, NB, DC], F32)
        xin_r = xin[:, :].rearrange("(n p) d -> p n d", p=128)
        for qq in range(3):
            nc.scalar.dma_start(out=x_s[:, 4 * qq:4 * qq + 4, :],
                                in_=xin_r[:, 4 * qq:4 * qq + 4, :])
        vp = sing.tile([128, DC], F32)
        nc.sync.dma_start(out=vp, in_=vp_d[:, :])
        sqk = sing.tile([128, NH, 2], F32)
        nc.sync.dma_start(out=sqk, in_=sqk_d[:, :, :])
        ident = sing.tile([128, 128], F32)
        nc.sync.dma_start(out=ident, in_=ident_d[:, :])
        phcr = sing.tile([3, NH, 2, 128], F32R)
        nc.sync.dma_start(out=phcr, in_=phc_d[:, :, :, :].bitcast(F32R))
        trowr = sing.tile([3, S], F32R)
        nc.sync.dma_start(out=trowr, in_=trow_d[:, :].bitcast(F32R))
        nc.sync.dma_start(out=x_s[:, 12:16, :], in_=xin_r[:, 12:16, :])
        opr = sing.tile([128, DC], F32)
        nc.sync.dma_start(out=opr, in_=op_d[:, :])

        bz = sing.tile([128, 1], F32)
        nc.vector.memset(bz, 0.0)
        onat = sing.tile([128, NB, DC], F32)
        vaug = []
        for j in range(NH):
            t = sing.tile([128, NB, DH + 1], BF16, tag=f"vaug{j}")
            nc.vector.memset(t[:, :, DH:DH + 1], INV_SQRT2)
            vaug.append(t)



        QT, KT = [None] * NH, [None] * NH
        # ACT-stream phase tracking: the Tile scheduler reorders freely and
        # does not model activation-table loads; explicit deps pin a
        # 5-phase order (sins v+h0, exps h0, sins h1-3, exps h1-3, out sins)
        # so head 0's exps start as soon as its Q/K are ready.
        sins, exps, osins = [], [], []

        # ---------------- V (4 quarters) ----------------
        sv_tiles = []

        def v_quarter(qq):
            xh = x_s[:, 4 * qq:4 * qq + 4, :]
            mv = rvpool.tile([128, 4, DC], F32, tag="rv")
            nc.vector._custom_dve(FRAC2, out=mv, in0=xh,
                                  in1=_dup_mid(vp[:, :], 4), s0=c_v, s1=MAGIC)
            sv = svpool.tile([128, 4, DC], BF16, tag="sv")
            sins.append(nc.scalar.activation(out=sv, in_=mv, func=AF.Sin,
                                             bias=bz[:, 0:1], scale=TWO_PI))
            sv_tiles.append((qq, sv))

        def v_scatter(qq, sv):
            for j in range(NH):
                nc.vector.tensor_copy(out=vaug[j][:, 4 * qq:4 * qq + 4, 0:DH],
                                      in_=sv[:, :, DH * j:DH * j + DH])

        # ---------------- QK prep ----------------
        def x2_dup(j, quarters=(slice(0, NB),)):
            """x2[:, n, 0:64] = x2[:, n, 64:128] = x_s[:, n, 64j:64j+64]."""
            x2 = x2pool.tile([128, NB, 128], F32, tag="x2")
            for qs in quarters:
                for half in range(2):
                    nc.gpsimd.tensor_copy(
                        out=x2[:, qs, 64 * half:64 * half + 64],
                        in_=x_s[:, qs, DH * j:DH * j + DH])
            return x2

        def qk_prep(j, x2, mid_cb=None, percc=False):
            """Per cc: rank-2 q phase, then transposes of the dup'd x block;
            FRAC + Sin per proj, with a rank-2 (k - q) delta matmul
            retargeting the PSUM to k. percc interleaves the k half per cc;
            otherwise the whole q projection finishes first (earlier q sin).
            mid_cb emits between the halves (DVE-stream ordering control)."""
            mq = mid.tile([128, S], F32, tag="mid")
            mk = mid.tile([128, S], F32, tag="mid")
            tiles = []
            for cc in range(4):
                xq = pprep.tile([128, 512], F32, tag="prep")
                sl = slice(512 * cc, 512 * cc + 512)
                nc.tensor.matmul(xq, phcr[:, j, 0, :], trowr[:, sl],
                                 start=True, stop=True)
                for sb in range(4):
                    n = 4 * cc + sb
                    nc.tensor.matmul(xq[:, 128 * sb:128 * sb + 128],
                                     x2[:, n, :], ident,
                                     is_transpose=True, start=False, stop=True,
                                     skip_group_check=True)
                nc.vector._custom_dve(FRAC, out=mq[:, sl], in0=xq,
                                      s0=sqk[:, j, 0:1], s1=0.0, imm2=MAGIC)
                if percc:
                    nc.tensor.matmul(xq, phcr[:, j, 1, :], trowr[:, sl],
                                     start=False, stop=True,
                                     skip_group_check=True)
                    nc.vector._custom_dve(FRAC, out=mk[:, sl], in0=xq,
                                          s0=sqk[:, j, 1:2], s1=0.0, imm2=MAGIC)
                    if cc == 1 and mid_cb is not None:
                        mid_cb()
                else:
                    tiles.append((xq, sl))
            if not percc:
                if mid_cb is not None:
                    mid_cb()
                for (xq, sl) in tiles:
                    nc.tensor.matmul(xq, phcr[:, j, 1, :], trowr[:, sl],
                                     start=False, stop=True,
                                     skip_group_check=True)
                    nc.vector._custom_dve(FRAC, out=mk[:, sl], in0=xq,
                                          s0=sqk[:, j, 1:2], s1=0.0, imm2=MAGIC)
            # head 0's sins split in 1024-col halves: ACT starts them as soon
            # as the first two FRACs land, unblocking the first score pack
            nsp = 2 if j == 0 else 1
            tq = qkpool.tile([128, S], BF16, tag="qk")
            for h in range(nsp):
                hs = slice(S // nsp * h, S // nsp * (h + 1))
                sins.append(nc.scalar.activation(out=tq[:, hs], in_=mq[:, hs],
                                                 func=AF.Sin, bias=bz[:, 0:1],
                                                 scale=TWO_PI))
            QT[j] = tq
            tk = qkpool.tile([128, S], BF16, tag="qk")
            for h in range(nsp):
                hs = slice(S // nsp * h, S // nsp * (h + 1))
                sins.append(nc.scalar.activation(out=tk[:, hs], in_=mk[:, hs],
                                                 func=AF.Sin, bias=bz[:, 0:1],
                                                 scale=TWO_PI))
            KT[j] = tk

        # ---------------- attention ----------------
        def attention(j, post_qc=None):
            for qc in range(4):
                ot_ps = pso.tile([65, 512], F32, tag="po")
                packs = _build_packs(qc)
                n_av = 4 * qc + 4
                avi = 0
                for pack in packs:
                    sc = psp.tile([128, 1024], F32, tag="ps")
                    for (kb, qs, N, off) in pack:
                        nc.tensor.matmul(sc[:, off:off + N],
                                         KT[j][:, 128 * kb:128 * kb + 128],
                                         QT[j][:, qs:qs + N],
                                         start=True, stop=True)
                    width = pack[-1][3] + pack[-1][2]
                    ext = expool.tile([128, 1024], BF16, tag="ex")
                    exps.append(nc.scalar.activation(
                        out=ext[:, 0:width], in_=sc[:, 0:width],
                        func=AF.Exp, bias=bz[:, 0:1], scale=EXP_SCALE))
                    for (kb, qs, N, off) in pack:
                        if kb >= 4 * qc:  # diagonal strip: zero exp where q < k
                            nc.gpsimd.affine_select(
                                out=ext[:, off:off + 128], in_=ext[:, off:off + 128],
                                pattern=[[1, 128]], compare_op=ALU.is_ge, fill=0.0,
                                base=0, channel_multiplier=-1)
                    for (kb, qs, N, off) in pack:
                        q0 = qs - 512 * qc
                        nc.tensor.matmul(ot_ps[:, q0:q0 + N],
                                         vaug[j][:, kb, :],
                                         ext[:, off:off + N],
                                         start=(avi == 0), stop=(avi == n_av - 1))
                        avi += 1
                ot_s = otpool.tile([65, 512], F32, tag="ot")
                nc.vector.tensor_copy(out=ot_s, in_=ot_ps)
                on_ps = psn.tile([128, 4, DH + 1], F32, tag="pn")
                for t4 in range(4):
                    nc.tensor.matmul(on_ps[:, t4, :],
                                     ot_s[:, 128 * t4:128 * t4 + 128],
                                     ident[0:65, 0:65],
                                     is_transpose=True, start=True, stop=True)
                den = tiny.tile([128, 4], F32, tag="tiny")
                nc.vector.reciprocal(out=den, in_=on_ps[:, :, DH:DH + 1])
                nc.vector.tensor_tensor(
                    out=onat[:, 4 * qc:4 * qc + 4, DH * j:DH * j + DH],
                    in0=on_ps[:, :, 0:DH], in1=_bcast_inner(den, DH),
                    op=ALU.mult)
                if post_qc is not None:
                    post_qc(qc)

        # ---------------- emission order ----------------
        # Pool: head-0 x2 copies per quarter as x lands, then later heads'.
        # DVE: v FRAC2s first (x-gated), then head-0 FRACs.
        v_quarter(0)
        v_quarter(1)
        v_quarter(2)
        x2_0 = x2_dup(0, quarters=[slice(4 * q, 4 * q + 4) for q in range(4)])
        qk_prep(0, x2_0, mid_cb=lambda: v_quarter(3), percc=False)
        qk_prep(1, x2_dup(1), percc=True)
        for qq, sv in sv_tiles:
            v_scatter(qq, sv)
        # x2 copies for heads 2/3 early in the Pool stream (before att0's
        # data-gated affines); their preps' PE/DVE work interleaves with
        # attention(0) via per-qc callbacks
        x2_2 = x2_dup(2)
        x2_3 = x2_dup(3)

        def prep_post_qc(qc):
            if qc == 0:
                qk_prep(2, x2_2, percc=True)
            elif qc == 2:
                qk_prep(3, x2_3, percc=True)

        attention(0, post_qc=prep_post_qc)
        nsin_a = 8            # v sins + head-0's 4 split sins: phase A
        nexp_h0 = len(exps)
        attention(1)
        attention(2)

        # out-stage FRAC2s interleave with attention(3): chunk data for qc
        # is complete right after head 3's qc normalize
        out_r = out_d[:, :].rearrange("(n p) d -> p n d", p=128)

        def emit_chunk(ci):
            b0, nb = OUT_CHUNKS[ci]
            rv = ropool.tile([128, nb, DC], F32, tag=f"ro{nb}")
            nc.vector._custom_dve(FRAC2, out=rv, in0=onat[:, b0:b0 + nb, :],
                                  in1=_dup_mid(opr[:, :], nb), s0=c_o, s1=MAGIC)
            osins.append(nc.scalar.activation(out=rv, in_=rv, func=AF.Sin,
                                              bias=bz[:, 0:1], scale=TWO_PI))
            nc.vector.tensor_scalar(out=rv, in0=rv, scalar1=SQRT2,
                                    scalar2=None, op0=ALU.mult)
            nc.sync.dma_start(out=out_r[:, b0:b0 + nb, :], in_=rv)

        def out_post_qc(qc):
            if qc < 3:
                emit_chunk(qc)
            else:
                emit_chunk(3)
                emit_chunk(4)

        attention(3, post_qc=out_post_qc)

        # phase-order bridges (A < B < C < D < E):
        #   A: v + h0 sins, B: h0 exps, C: h1-3 sins, D: h1-3 exps, E: out sins
        def bridge(prev_phase, next_phase):
            first = next_phase[0]
            for p_i in prev_phase:
                bass._add_dep_helper(first.ins, p_i.ins, sync=True,
                                     reason="act-table-order")
            for n_i in next_phase[1:]:
                bass._add_dep_helper(n_i.ins, first.ins, sync=True,
                                     reason="act-table-order")

        sins_a, sins_c = sins[:nsin_a], sins[nsin_a:]
        exps_b, exps_d = exps[:nexp_h0], exps[nexp_h0:]
        bridge(sins_a, exps_b)
        bridge(exps_b, sins_c)
        bridge(sins_c, exps_d)
        bridge(exps_d, osins)

    nc.finalize()
    return nc


def _host_params(inputs, c):
    """Per-core input dict for core c."""
    b, g = c // 4, c % 4
    inv2pi = 1.0 / (2.0 * np.pi)
    x = np.asarray(inputs["x"], dtype=np.float32)
    xin = np.ascontiguousarray(x[b, :, DC * g:DC * g + DC])

    def f64(a):
        return np.asarray(a, dtype=np.float64)

    def hi_lo(v):
        """Split f64 v into f32 hi (12 mantissa bits, so hi*t is exact in
        f32r for t < 2^12) + f32 lo remainder."""
        m, e = np.frexp(v)
        hi = np.ldexp(np.round(m * 2**12) / 2**12, e).astype(np.float32)
        lo = (v - hi.astype(np.float64)).astype(np.float32)
        return hi, lo

    rows = np.arange(128) % DH
    cos_row = (np.arange(128) < DH).astype(np.float64) * 0.25
    sqk = np.zeros((128, NH, 2), dtype=np.float32)
    phc = np.zeros((3, NH, 2, 128), dtype=np.float32)
    for j in range(NH):
        h = NH * g + j
        ph_s = {}
        for pi, (wn, bn, pn) in enumerate([("w_q", "b_q", "phi_q"),
                                           ("w_k", "b_k", "phi_k")]):
            w = f64(inputs[wn])[h]
            bb = f64(inputs[bn])[h]
            ph = f64(inputs[pn])[h]
            s = (inv2pi / (1.0 + np.abs(w)))[rows]
            phi2 = (ph * inv2pi)[rows] / s
            c2 = ((bb * inv2pi)[rows] + cos_row) / s
            sqk[:, j, pi] = s
            ph_s[pi] = (phi2, c2)
        phq_hi, phq_lo = hi_lo(ph_s[0][0])
        phk_hi, _ = hi_lo(ph_s[1][0])
        dphi_hi = (phk_hi - phq_hi).astype(np.float64)
        dphi_lo = (ph_s[1][0] - ph_s[0][0] - dphi_hi).astype(np.float32)
        phc[0, j, 0, :] = phq_hi
        phc[1, j, 0, :] = phq_lo
        phc[2, j, 0, :] = ph_s[0][1]
        phc[0, j, 1, :] = dphi_hi                   # k - q phase delta
        phc[1, j, 1, :] = dphi_lo
        phc[2, j, 1, :] = ph_s[1][1] - ph_s[0][1]

    vp = np.zeros((128, DC), dtype=np.float32)
    wv = f64(inputs["w_v"])[NH * g:NH * g + NH].reshape(-1)
    vp[:, :] = (inv2pi / (1.0 + np.abs(wv)))[None, :]

    op = np.zeros((128, DC), dtype=np.float32)
    wo = f64(inputs["w_out"])[DC * g:DC * g + DC]
    op[:, :] = (inv2pi / (1.0 + np.abs(wo)))[None, :]

    return {"xin": xin, "sqk": sqk, "phc": phc, "vp": vp, "opar": op}


_NC_CACHE = {}


def kernel(**inputs) -> np.ndarray:
    in_maps = [_host_params(inputs, c) for c in range(8)]
    inv2pi = 1.0 / (2.0 * np.pi)
    bv = np.asarray(inputs["b_v"], dtype=np.float64).reshape(-1)
    bo = np.asarray(inputs["b_out"], dtype=np.float64).reshape(-1)
    assert np.all(bv == bv[0]) and np.all(bo == bo[0]), "non-uniform b_v/b_out unsupported"
    c_v = float(np.float32(bv[0] * inv2pi + 0.125))
    c_o = float(np.float32(bo[0] * inv2pi + 0.125))
    key = (c_v, c_o)
    if _NC_CACHE.get("key") != key:
        _NC_CACHE["nc"] = build_nc(c_v, c_o)
        _NC_CACHE["key"] = key
    nc = _NC_CACHE["nc"]
    res = run_bass_kernel_spmd(nc, in_maps, core_ids=list(range(8)))
    full = np.empty((B, S, D), dtype=np.float32)
    for c in range(8):
        b, g = c // 4, c % 4
        full[b, :, DC * g:DC * g + DC] = res.results[c]["out"]
    return full
